# revision 1
# baseline (speedup 1.0000x reference)
"""GroupedQueryAttention forward on 8 Trainium2 NeuronCores (Bass/Tile).

Sharding (per spec hint): data-parallel over batch (B=2) x tensor-parallel
over KV-head groups (4 groups of 2 KV heads + their 8 query heads each).
Core c -> (batch b = c // 4, group g = c % 4).

Each core computes, for its batch element and its 8 query heads:
  qT/kT projections in transposed layout (lhsT = W, rhs = xT), V natural via
  on-chip PE transpose of vT; causal softmax without max-subtraction (scores
  are ~N(0,1) after the 1/sqrt(hd) scale, exp cannot overflow); the softmax
  denominator is produced by the same matmul as attn@V via a ones-column
  appended to V; normalization is folded into the o_proj stationary tiles.
  o_proj is row-parallel: each core emits a full [N, D] fp32 partial, and the
  host sums the 4 partials per batch element (the "all-reduce" of the o_proj).

All device compute is bf16 with fp32 PSUM accumulation. The host pre-casts
and pre-transposes x (xT) and pre-slices/reorders the weight shards so the
device performs no layout work on the inputs.
"""

import numpy as np

import concourse.bass as bass  # noqa: F401  (import keeps engine registry warm)
import concourse.mybir as mybir
import concourse.tile as tile
from concourse import bacc, bass_utils

# Problem shape (hardcoded per contract).
B, N, D = 2, 2048, 2048
NUM_HEADS = 32
NUM_KV_HEADS = 8
HD = 64                      # head dim
G = NUM_HEADS // NUM_KV_HEADS  # 4 query heads per kv head
N_CORES = 8
LQ = 8                       # local query heads per core (2 kv heads * G)
NT = D // 128                # 16 contraction tiles
NCHUNK = 4                   # token chunks of 512
CH = 512

_CACHE = {}


def _build():
    nc = bacc.Bacc("TRN2", target_bir_lowering=False, debug=False,
                   num_devices=N_CORES)
    f32, bf16 = mybir.dt.float32, mybir.dt.bfloat16

    xT = nc.dram_tensor("xT", [D, N], bf16, kind="ExternalInput")
    wq = nc.dram_tensor("wq", [D, 512], bf16, kind="ExternalInput")
    wk = nc.dram_tensor("wk", [D, 128], bf16, kind="ExternalInput")
    wv = nc.dram_tensor("wv", [D, 128], bf16, kind="ExternalInput")
    wo = nc.dram_tensor("wo", [512, D], bf16, kind="ExternalInput")
    msk = nc.dram_tensor("msk", [128, 4 * CH], bf16, kind="ExternalInput")
    iden = nc.dram_tensor("iden", [128, 128], bf16, kind="ExternalInput")
    sel = nc.dram_tensor("sel", [8, 4 * 128], f32, kind="ExternalInput")
    part = nc.dram_tensor("part", [N, D], f32, kind="ExternalOutput")

    with tile.TileContext(nc) as tc:
        with (
            tc.tile_pool(name="const", bufs=1) as cpool,
            tc.tile_pool(name="proj", bufs=1) as ppool,
            tc.tile_pool(name="work", bufs=4) as wpool,
            tc.tile_pool(name="att", bufs=1) as apool,
            tc.tile_pool(name="stage", bufs=3) as spool,
            tc.tile_pool(name="ps_s", bufs=2, space="PSUM") as ps_s,
            tc.tile_pool(name="ps_av", bufs=4, space="PSUM") as ps_av,
            tc.tile_pool(name="ps_m", bufs=1, space="PSUM") as ps_m,
        ):
            # ---- load constants / inputs to SBUF -------------------------
            xt = cpool.tile([128, NT * N], bf16, tag="xt")
            nc.sync.dma_start(
                xt[:].rearrange("p (t n) -> p t n", t=NT),
                xT.ap().rearrange("(t p) n -> p t n", p=128))
            wq_t = cpool.tile([128, NT * 512], bf16, tag="wq")
            nc.sync.dma_start(
                wq_t[:].rearrange("p (t o) -> p t o", t=NT),
                wq.ap().rearrange("(t p) o -> p t o", p=128))
            wk_t = cpool.tile([128, NT * 128], bf16, tag="wk")
            nc.sync.dma_start(
                wk_t[:].rearrange("p (t o) -> p t o", t=NT),
                wk.ap().rearrange("(t p) o -> p t o", p=128))
            wv_t = cpool.tile([128, NT * 128], bf16, tag="wv")
            nc.sync.dma_start(
                wv_t[:].rearrange("p (t o) -> p t o", t=NT),
                wv.ap().rearrange("(t p) o -> p t o", p=128))
            wo_t = cpool.tile([128, 4 * D], bf16, tag="wo")
            nc.sync.dma_start(
                wo_t[:].rearrange("p (t o) -> p t o", t=4),
                wo.ap().rearrange("(t p) o -> p t o", p=128))
            msk_t = cpool.tile([128, 4 * CH], bf16, tag="msk")
            nc.sync.dma_start(msk_t[:], msk.ap()[:])
            id_t = cpool.tile([128, 128], bf16, tag="iden")
            nc.sync.dma_start(id_t[:], iden.ap()[:])
            ones64 = cpool.tile([1, 64], f32, tag="ones64")
            nc.vector.memset(ones64[:], 1.0)
            sel_t = cpool.tile([8, 4 * 128], f32, tag="sel")
            nc.sync.dma_start(sel_t[:], sel.ap()[:])

            # ---- projections --------------------------------------------
            # kT2 [128 (2 kv heads x 64), N]
            kt2 = ppool.tile([128, N], bf16, tag="kt2")
            for j in range(N // CH):
                ps = ps_m.tile([128, CH], f32, tag="misc")
                for t in range(NT):
                    nc.tensor.matmul(
                        ps[:], wk_t[:, t * 128:(t + 1) * 128],
                        xt[:, t * N + j * CH: t * N + (j + 1) * CH],
                        start=(t == 0), stop=(t == NT - 1))
                nc.scalar.activation(kt2[:, j * CH:(j + 1) * CH], ps[:],
                                     mybir.ActivationFunctionType.Copy)
            # vT [128, N] then transpose to V3 [128, 16*130] (V + ones col)
            v3 = apool.tile([128, 16 * 130], bf16, tag="v3")
            nc.vector.memset(v3[:], 1.0)
            for j in range(N // CH):
                ps = ps_m.tile([128, CH], f32, tag="misc")
                for t in range(NT):
                    nc.tensor.matmul(
                        ps[:], wv_t[:, t * 128:(t + 1) * 128],
                        xt[:, t * N + j * CH: t * N + (j + 1) * CH],
                        start=(t == 0), stop=(t == NT - 1))
                vt_s = spool.tile([128, CH], bf16, tag="vt")
                nc.scalar.activation(vt_s[:], ps[:],
                                     mybir.ActivationFunctionType.Copy)
                for s in range(4):       # 4 m-tiles of 128 in this chunk
                    mt = 4 * j + s
                    pst = ps_m.tile([128, 128], bf16, tag="tr")
                    nc.tensor.transpose(pst[:], vt_s[:, s * 128:(s + 1) * 128],
                                        id_t[:])
                    nc.vector.tensor_copy(v3[:, mt * 130: mt * 130 + 64],
                                          pst[:, 0:64])
                    nc.vector.tensor_copy(v3[:, mt * 130 + 65: mt * 130 + 129],
                                          pst[:, 64:128])
            # qT2 chunks a=0..3: [128 (head a | head a+4), N]
            qt2 = []
            for a in range(4):
                qa = ppool.tile([128, N], bf16, tag=f"qt2_{a}")
                for j in range(N // CH):
                    ps = ps_m.tile([128, CH], f32, tag="misc")
                    for t in range(NT):
                        nc.tensor.matmul(
                            ps[:], wq_t[:, t * 512 + a * 128: t * 512 + (a + 1) * 128],
                            xt[:, t * N + j * CH: t * N + (j + 1) * CH],
                            start=(t == 0), stop=(t == NT - 1))
                    nc.scalar.activation(qa[:, j * CH:(j + 1) * CH], ps[:],
                                         mybir.ActivationFunctionType.Copy)
                qt2.append(qa)

            # ---- attention + o_proj per token chunk ---------------------
            for ci in range(NCHUNK):
                n0 = ci * CH
                mt_hi = 4 * ci + 4          # m-tiles 0..mt_hi-1
                aot = []                     # attn_outT tiles per pair
                sum8 = apool.tile([1, 8 * CH], f32, tag="sum8")
                for wave in range(2):
                    for a in (2 * wave, 2 * wave + 1):
                        pa0 = ps_av.tile([128, CH], f32, tag="av")
                        pa1 = ps_av.tile([128, CH], f32, tag="av")
                        for mt in range(mt_hi):
                            diag = mt - 4 * ci
                            ss0 = ps_s.tile([128, CH], f32, tag="s")
                            ss1 = ps_s.tile([128, CH], f32, tag="s")
                            nc.tensor.matmul(
                                ss0[:], kt2[0:64, mt * 128:(mt + 1) * 128],
                                qt2[a][0:64, n0:n0 + CH],
                                start=True, stop=True)
                            nc.tensor.matmul(
                                ss1[:], kt2[64:128, mt * 128:(mt + 1) * 128],
                                qt2[a][64:128, n0:n0 + CH],
                                start=True, stop=True)
                            pt0 = wpool.tile([128, CH], bf16, tag="pt")
                            pt1 = wpool.tile([128, CH], bf16, tag="pt")
                            nc.scalar.activation(
                                pt0[:], ss0[:],
                                mybir.ActivationFunctionType.Exp, scale=0.125)
                            nc.scalar.activation(
                                pt1[:], ss1[:],
                                mybir.ActivationFunctionType.Exp, scale=0.125)
                            if diag >= 0:
                                mslc = msk_t[:, diag * CH:(diag + 1) * CH]
                                nc.vector.tensor_mul(pt0[:], pt0[:], mslc)
                                nc.vector.tensor_mul(pt1[:], pt1[:], mslc)
                            nc.tensor.matmul(
                                pa0[0:65, :], v3[:, mt * 130: mt * 130 + 65],
                                pt0[:], start=(mt == 0), stop=(mt == mt_hi - 1))
                            nc.tensor.matmul(
                                pa1[0:65, :], v3[:, mt * 130 + 65: mt * 130 + 130],
                                pt1[:], start=(mt == 0), stop=(mt == mt_hi - 1))
                        ao = apool.tile([128, CH], bf16, tag=f"ao_{a}")
                        nc.vector.tensor_copy(ao[0:64, :], pa0[0:64, :])
                        nc.vector.tensor_copy(ao[64:128, :], pa1[0:64, :])
                        nc.vector.tensor_copy(sum8[0:1, a * CH:(a + 1) * CH], pa0[64:65, :])
                        nc.vector.tensor_copy(sum8[0:1, (a + 4) * CH:(a + 5) * CH], pa1[64:65, :])
                        aot.append(ao)
                aos = []
                for a in range(4):
                    rb = ps_m.tile([128, CH], f32, tag="misc")
                    nc.tensor.matmul(rb[0:64, :], ones64[0:1, :],
                                     sum8[0:1, a * CH:(a + 1) * CH],
                                     start=True, stop=True, tile_position=(0, 0))
                    nc.tensor.matmul(rb[64:128, :], ones64[0:1, :],
                                     sum8[0:1, (a + 4) * CH:(a + 5) * CH],
                                     start=True, stop=True, tile_position=(0, 64))
                    rbr = spool.tile([128, CH], f32, tag="rbr")
                    nc.vector.reciprocal(rbr[:], rb[:])
                    an = apool.tile([128, CH], bf16, tag=f"aos_{a}")
                    nc.vector.tensor_mul(an[:], aot[a][:], rbr[:])
                    aos.append(an)
                # o_proj: out[n, :] += sum_c attn_outT_s[c, n] * Wo[c, :]
                for nt in range(4):
                    for dc in range(4):
                        po = ps_m.tile([128, CH], f32, tag="misc")
                        for a in range(4):
                            nc.tensor.matmul(
                                po[:], aos[a][:, nt * 128:(nt + 1) * 128],
                                wo_t[:, a * D + dc * CH: a * D + (dc + 1) * CH],
                                start=(a == 0), stop=(a == 3))
                        st = spool.tile([128, CH], f32, tag="ost")
                        nc.vector.tensor_copy(st[:], po[:])
                        nc.sync.dma_start(
                            part.ap()[n0 + nt * 128: n0 + (nt + 1) * 128,
                                      dc * CH:(dc + 1) * CH],
                            st[:])
    nc.compile()
    return nc


def _prep_in_maps(x, Wq, Wk, Wv, Wo):
    import jax.numpy as jnp

    def to_bf16(a):
        return np.asarray(jnp.asarray(np.asarray(a), dtype=jnp.bfloat16))

    # causal mask tiles for diagonal offsets 0..3 (within a 512 chunk)
    msk = np.zeros((128, 4 * CH), np.float32)
    for k in range(4):
        i = np.arange(128)[:, None]
        j = np.arange(CH)[None, :]
        msk[:, k * CH:(k + 1) * CH] = (i + 128 * k <= j).astype(np.float32)
    iden = np.eye(128, dtype=np.float32)
    sel = np.zeros((8, 4 * 128), np.float32)
    for a in range(4):
        sel[a, a * 128: a * 128 + 64] = 1.0
        sel[a + 4, a * 128 + 64: (a + 1) * 128] = 1.0

    in_maps = []
    for c in range(N_CORES):
        b, g = c // 4, c % 4
        qh = [8 * g + a for a in range(8)]      # global q heads for this core
        # Wq columns reordered into pair chunks [head a | head a+4]
        wq_cols = []
        for a in range(4):
            wq_cols.append(np.arange(qh[a] * HD, (qh[a] + 1) * HD))
            wq_cols.append(np.arange(qh[a + 4] * HD, (qh[a + 4] + 1) * HD))
        wq_r = np.asarray(Wq)[:, np.concatenate(wq_cols)]
        wo_rows = wq_cols  # same ordering for Wo rows
        wo_r = np.asarray(Wo)[np.concatenate(wo_rows), :]
        wk_s = np.asarray(Wk)[:, 2 * g * HD: (2 * g + 2) * HD]
        wv_s = np.asarray(Wv)[:, 2 * g * HD: (2 * g + 2) * HD]
        in_maps.append({
            "xT": to_bf16(np.asarray(x)[b].T),
            "wq": to_bf16(wq_r),
            "wk": to_bf16(wk_s),
            "wv": to_bf16(wv_s),
            "wo": to_bf16(wo_r),
            "msk": to_bf16(msk),
            "iden": to_bf16(iden),
            "sel": sel,
        })
    return in_maps


def kernel(x, Wq, Wk, Wv, Wo, trace=False):
    if "nc" not in _CACHE:
        _CACHE["nc"] = _build()
    nc = _CACHE["nc"]
    in_maps = _prep_in_maps(x, Wq, Wk, Wv, Wo)
    res = bass_utils.run_bass_kernel_spmd(
        nc, in_maps, core_ids=list(range(N_CORES)), trace=trace)
    _CACHE["last_result"] = res
    out = np.zeros((B, N, D), np.float32)
    for c in range(N_CORES):
        out[c // 4] += res.results[c]["part"]
    return out



# revision 14
# speedup vs baseline: 1.5715x; 1.5715x over previous
"""GroupedQueryAttention forward on 8 Trainium2 NeuronCores (Bass/Tile).

Sharding (per spec hint): data-parallel over batch (B=2) x tensor-parallel
over KV-head groups (4 groups of 2 KV heads + their 8 query heads each).
Core c -> (batch b = c // 4, group g = c % 4).

Each core computes, for its batch element and its 8 query heads:
  qT/kT projections in transposed layout (lhsT = W, rhs = xT), V natural via
  on-chip PE transpose of vT; causal softmax without max-subtraction (scores
  are ~N(0,1) after the 1/sqrt(hd) scale, exp cannot overflow); the softmax
  denominator is produced by the same matmul as attn@V via a ones-column
  appended to V.
  o_proj is row-parallel: each core emits a full [N, D] fp32 partial, and the
  host sums the 4 partials per batch element (the "all-reduce" of the o_proj).

Perf structure vs the v1 kernel:
  - the two 64-contraction score matmuls of a head pair write one 2-bank
    PSUM tile [128,1024] and are issued adjacently so the PE row-group
    tiling (h0 rows 0-63 / h64 rows 64-127) runs them concurrently;
  - exp runs once per (pair, mt) over [128,1024] (halves ACT instruction
    overhead), mask is one [128,1024] DVE multiply;
  - softmax normalization uses reciprocal_approx_fast and is fused into
    the PSUM->SBUF attn-out copy;
  - xT is DMA'd in 4 column windows so projections start immediately.

All device compute is bf16 with fp32 PSUM accumulation. The host pre-casts
and pre-transposes x (xT) and pre-slices/reorders the weight shards so the
device performs no layout work on the inputs.
"""

import numpy as np

import concourse.bass as bass  # noqa: F401  (import keeps engine registry warm)
import concourse.mybir as mybir
import concourse.tile as tile
from concourse import bacc, bass_utils

# Problem shape (hardcoded per contract).
B, N, D = 2, 2048, 2048
NUM_HEADS = 32
NUM_KV_HEADS = 8
HD = 64                      # head dim
G = NUM_HEADS // NUM_KV_HEADS  # 4 query heads per kv head
N_CORES = 8
LQ = 8                       # local query heads per core (2 kv heads * G)
NT = D // 128                # 16 contraction tiles
NCHUNK = 4                   # token chunks of 512
CH = 512

_CACHE = {}


def _build():
    nc = bacc.Bacc("TRN2", target_bir_lowering=False, debug=False,
                   num_devices=N_CORES)
    f32, bf16 = mybir.dt.float32, mybir.dt.bfloat16

    xT = nc.dram_tensor("xT", [D, N], bf16, kind="ExternalInput")
    wq = nc.dram_tensor("wq", [D, 512], bf16, kind="ExternalInput")
    wk = nc.dram_tensor("wk", [D, 128], bf16, kind="ExternalInput")
    wv = nc.dram_tensor("wv", [D, 128], bf16, kind="ExternalInput")
    wo = nc.dram_tensor("wo", [512, D], bf16, kind="ExternalInput")
    msk = nc.dram_tensor("msk", [128, 4 * 2 * CH], bf16, kind="ExternalInput")
    part = nc.dram_tensor("part", [N, D], f32, kind="ExternalOutput")

    with tile.TileContext(nc) as tc:
        with (
            tc.tile_pool(name="const", bufs=1) as cpool,
            tc.tile_pool(name="proj", bufs=1) as ppool,
            tc.tile_pool(name="work", bufs=4) as wpool,
            tc.tile_pool(name="att", bufs=1) as apool,
            tc.tile_pool(name="stage", bufs=3) as spool,
            tc.tile_pool(name="ps_s", bufs=2, space="PSUM") as ps_s,
            tc.tile_pool(name="ps_av", bufs=1, space="PSUM") as ps_av,
            tc.tile_pool(name="ps_m", bufs=2, space="PSUM") as ps_m,
        ):
            # ---- load constants / inputs to SBUF -------------------------
            # xT in 4 column windows so the first projection matmuls can
            # start as soon as window 0 lands.
            xt = cpool.tile([128, NT * N], bf16, tag="xt")
            for j in range(4):
                nc.sync.dma_start(
                    xt[:].rearrange("p (t n) -> p t n", t=NT)[:, :, j * CH:(j + 1) * CH],
                    xT.ap().rearrange("(t p) n -> p t n", p=128)[:, :, j * CH:(j + 1) * CH])
            wk_t = cpool.tile([128, NT * 128], bf16, tag="wk")
            nc.sync.dma_start(
                wk_t[:].rearrange("p (t o) -> p t o", t=NT),
                wk.ap().rearrange("(t p) o -> p t o", p=128))
            wv_t = cpool.tile([128, NT * 128], bf16, tag="wv")
            nc.sync.dma_start(
                wv_t[:].rearrange("p (t o) -> p t o", t=NT),
                wv.ap().rearrange("(t p) o -> p t o", p=128))
            wq_t = cpool.tile([128, NT * 512], bf16, tag="wq")
            nc.sync.dma_start(
                wq_t[:].rearrange("p (t o) -> p t o", t=NT),
                wq.ap().rearrange("(t p) o -> p t o", p=128))
            wo_t = cpool.tile([128, 4 * D], bf16, tag="wo")
            nc.sync.dma_start(
                wo_t[:].rearrange("p (t o) -> p t o", t=4),
                wo.ap().rearrange("(t p) o -> p t o", p=128))
            msk_t = cpool.tile([128, 4 * 2 * CH], bf16, tag="msk")
            nc.sync.dma_start(msk_t[:], msk.ap()[:])
            ones64 = cpool.tile([1, 64], f32, tag="ones64")
            nc.vector.memset(ones64[:], 1.0)

            # ---- projections --------------------------------------------
            # kT2 [128 (2 kv heads x 64), N]
            kt2 = ppool.tile([128, N], bf16, tag="kt2")
            for j in range(N // CH):
                ps = ps_m.tile([128, CH], f32, tag="misc")
                for t in range(NT):
                    nc.tensor.matmul(
                        ps[:], wk_t[:, t * 128:(t + 1) * 128],
                        xt[:, t * N + j * CH: t * N + (j + 1) * CH],
                        start=(t == 0), stop=(t == NT - 1))
                nc.scalar.activation(kt2[:, j * CH:(j + 1) * CH], ps[:],
                                     mybir.ActivationFunctionType.Copy)
            # vT [128, N] then DMA-transpose to V3 [128, 16*130].
            # Per m-tile layout: 0:64 = V_h0, 64 = ones, 65:129 = V_h1,
            # 129 = ones (the ones columns produce the softmax denominators
            # in row 64 of the attn@V PSUM output).
            v3 = apool.tile([128, 16 * 130], bf16, tag="v3")
            nc.vector.memset(v3[:], 1.0)
            for j in range(N // CH):
                ps = ps_m.tile([128, CH], f32, tag="misc")
                for t in range(NT):
                    nc.tensor.matmul(
                        ps[:], wv_t[:, t * 128:(t + 1) * 128],
                        xt[:, t * N + j * CH: t * N + (j + 1) * CH],
                        start=(t == 0), stop=(t == NT - 1))
                vt_s = spool.tile([128, CH], bf16, tag="vt")
                nc.scalar.activation(vt_s[:], ps[:],
                                     mybir.ActivationFunctionType.Copy)
                for s in range(4):       # 4 m-tiles of 128 in this chunk
                    mt = 4 * j + s
                    # full-tile DMA transpose (partition-offset inputs are
                    # broken in the xbar path), then split around the ones
                    # column with two free-dim-offset DVE copies.
                    vtr = spool.tile([128, 128], bf16, tag="vtr")
                    nc.sync.dma_start_transpose(
                        vtr[:], vt_s[:, s * 128:(s + 1) * 128])
                    nc.vector.tensor_copy(v3[:, mt * 130: mt * 130 + 64],
                                          vtr[:, 0:64])
                    nc.vector.tensor_copy(v3[:, mt * 130 + 65: mt * 130 + 129],
                                          vtr[:, 64:128])
            # qT2 chunks a=0..3: [128 (head a | head a+4), N]
            qt2 = []
            for a in range(4):
                qa = ppool.tile([128, N], bf16, tag=f"qt2_{a}")
                for j in range(N // CH):
                    ps = ps_m.tile([128, CH], f32, tag="misc")
                    for t in range(NT):
                        nc.tensor.matmul(
                            ps[:], wq_t[:, t * 512 + a * 128: t * 512 + (a + 1) * 128],
                            xt[:, t * N + j * CH: t * N + (j + 1) * CH],
                            start=(t == 0), stop=(t == NT - 1))
                    nc.scalar.activation(qa[:, j * CH:(j + 1) * CH], ps[:],
                                         mybir.ActivationFunctionType.Copy)
                qt2.append(qa)

            # ---- attention + o_proj per token chunk ---------------------
            for ci in range(NCHUNK):
                n0 = ci * CH
                mt_hi = 4 * ci + 4          # m-tiles 0..mt_hi-1
                aos = []                     # normalized attn outs per pair
                for a in range(4):
                    # pa: [0:64, 0:512]=head a+4 out, [0:64,512:1024]=head a
                    # out, row 64 of each half = softmax denominators.
                    # (h64 half goes in bank 0: walrus rejects tile_position
                    # row 64 combined with a non-zero PSUM output offset.)
                    pa = ps_av.tile([128, 2 * CH], f32, tag="av")
                    for mt in range(mt_hi):
                        diag = mt - 4 * ci
                        ss = ps_s.tile([128, 2 * CH], f32, tag="s")
                        # two 64-contraction matmuls on distinct PE row
                        # groups (h64 / h0), issued adjacently -> concurrent
                        nc.tensor.matmul(
                            ss[:, 0:CH], kt2[64:128, mt * 128:(mt + 1) * 128],
                            qt2[a][64:128, n0:n0 + CH],
                            start=True, stop=True)
                        nc.tensor.matmul(
                            ss[:, CH:2 * CH], kt2[0:64, mt * 128:(mt + 1) * 128],
                            qt2[a][0:64, n0:n0 + CH],
                            start=True, stop=True)
                        pt = wpool.tile([128, 2 * CH], bf16, tag="pt")
                        nc.scalar.activation(
                            pt[:], ss[:],
                            mybir.ActivationFunctionType.Exp, scale=0.125)
                        if diag >= 0:
                            nc.vector.tensor_mul(
                                pt[:], pt[:],
                                msk_t[:, diag * 2 * CH:(diag + 1) * 2 * CH])
                        nc.tensor.matmul(
                            pa[0:65, 0:CH],
                            v3[:, mt * 130 + 65: mt * 130 + 130],
                            pt[:, 0:CH],
                            start=(mt == 0), stop=(mt == mt_hi - 1))
                        nc.tensor.matmul(
                            pa[0:65, CH:2 * CH],
                            v3[:, mt * 130: mt * 130 + 65],
                            pt[:, CH:2 * CH],
                            start=(mt == 0), stop=(mt == mt_hi - 1))
                    # normalization: denominators -> broadcast -> approx
                    # reciprocal -> fused into the PSUM->SBUF copy.
                    sm = spool.tile([1, 2 * CH], f32, tag="sm")
                    nc.vector.tensor_copy(sm[:], pa[64:65, :])
                    rb = ps_m.tile([128, CH], f32, tag="misc")
                    nc.tensor.matmul(rb[0:64, :], ones64[0:1, :],
                                     sm[0:1, CH:2 * CH],
                                     start=True, stop=True, tile_position=(0, 0))
                    nc.tensor.matmul(rb[64:128, :], ones64[0:1, :],
                                     sm[0:1, 0:CH],
                                     start=True, stop=True, tile_position=(0, 64))
                    rbr = spool.tile([128, CH], f32, tag="rbr")
                    nc.vector.reciprocal_approx_fast(rbr[:], rb[:])
                    an = apool.tile([128, CH], bf16, tag=f"aos_{a}")
                    nc.vector.tensor_mul(an[0:64, :], pa[0:64, CH:2 * CH],
                                         rbr[0:64, :])
                    nc.vector.tensor_mul(an[64:128, :], pa[0:64, 0:CH],
                                         rbr[64:128, :])
                    aos.append(an)
                # o_proj: out[n, :] += sum_c attn_outT_s[c, n] * Wo[c, :]
                for nt in range(4):
                    for dc in range(4):
                        po = ps_m.tile([128, CH], f32, tag="misc")
                        for a in range(4):
                            nc.tensor.matmul(
                                po[:], aos[a][:, nt * 128:(nt + 1) * 128],
                                wo_t[:, a * D + dc * CH: a * D + (dc + 1) * CH],
                                start=(a == 0), stop=(a == 3))
                        st = spool.tile([128, CH], f32, tag="ost")
                        nc.vector.tensor_copy(st[:], po[:])
                        nc.sync.dma_start(
                            part.ap()[n0 + nt * 128: n0 + (nt + 1) * 128,
                                      dc * CH:(dc + 1) * CH],
                            st[:])
    nc.compile()
    return nc


def _prep_in_maps(x, Wq, Wk, Wv, Wo):
    import jax.numpy as jnp

    def to_bf16(a):
        return np.asarray(jnp.asarray(np.asarray(a), dtype=jnp.bfloat16))

    # causal mask tiles for diagonal offsets 0..3 (within a 512 chunk),
    # duplicated at [d*1024 : d*1024+512] and [+512 : +1024] so one DVE
    # multiply covers both head-halves of a [128,1024] pt tile.
    msk = np.zeros((128, 4 * 2 * CH), np.float32)
    for k in range(4):
        i = np.arange(128)[:, None]
        j = np.arange(CH)[None, :]
        m = (i + 128 * k <= j).astype(np.float32)
        msk[:, k * 2 * CH: k * 2 * CH + CH] = m
        msk[:, k * 2 * CH + CH: (k + 1) * 2 * CH] = m

    in_maps = []
    for c in range(N_CORES):
        b, g = c // 4, c % 4
        qh = [8 * g + a for a in range(8)]      # global q heads for this core
        # Wq columns reordered into pair chunks [head a | head a+4]
        wq_cols = []
        for a in range(4):
            wq_cols.append(np.arange(qh[a] * HD, (qh[a] + 1) * HD))
            wq_cols.append(np.arange(qh[a + 4] * HD, (qh[a + 4] + 1) * HD))
        wq_r = np.asarray(Wq)[:, np.concatenate(wq_cols)]
        wo_rows = wq_cols  # same ordering for Wo rows
        wo_r = np.asarray(Wo)[np.concatenate(wo_rows), :]
        wk_s = np.asarray(Wk)[:, 2 * g * HD: (2 * g + 2) * HD]
        wv_s = np.asarray(Wv)[:, 2 * g * HD: (2 * g + 2) * HD]
        in_maps.append({
            "xT": to_bf16(np.asarray(x)[b].T),
            "wq": to_bf16(wq_r),
            "wk": to_bf16(wk_s),
            "wv": to_bf16(wv_s),
            "wo": to_bf16(wo_r),
            "msk": to_bf16(msk),
        })
    return in_maps


def kernel(x, Wq, Wk, Wv, Wo, trace=False):
    if "nc" not in _CACHE:
        _CACHE["nc"] = _build()
    nc = _CACHE["nc"]
    in_maps = _prep_in_maps(x, Wq, Wk, Wv, Wo)
    res = bass_utils.run_bass_kernel_spmd(
        nc, in_maps, core_ids=list(range(N_CORES)), trace=trace)
    _CACHE["last_result"] = res
    out = np.zeros((B, N, D), np.float32)
    for c in range(N_CORES):
        out[c // 4] += res.results[c]["part"]
    return out


# revision 21
# speedup vs baseline: 1.6304x; 1.0375x over previous
"""GroupedQueryAttention forward on 8 Trainium2 NeuronCores (Bass/Tile).

Sharding (per spec hint): data-parallel over batch (B=2) x tensor-parallel
over KV-head groups (4 groups of 2 KV heads + their 8 query heads each).
Core c -> (batch b = c // 4, group g = c % 4).

Each core computes, for its batch element and its 8 query heads:
  qT/kT projections in transposed layout (lhsT = W, rhs = xT), V natural via
  on-chip PE transpose of vT; causal softmax without max-subtraction (scores
  are ~N(0,1) after the 1/sqrt(hd) scale, exp cannot overflow); the softmax
  denominator is produced by the same matmul as attn@V via a ones-column
  appended to V.
  o_proj is row-parallel: each core emits a full [N, D] fp32 partial, and the
  host sums the 4 partials per batch element (the "all-reduce" of the o_proj).

Perf structure vs the v1 kernel:
  - the two 64-contraction score matmuls of a head pair write one 2-bank
    PSUM tile [128,1024] and are issued adjacently so the PE row-group
    tiling (h0 rows 0-63 / h64 rows 64-127) runs them concurrently;
  - exp runs once per (pair, mt) over [128,1024] (halves ACT instruction
    overhead), mask is one [128,1024] DVE multiply;
  - softmax normalization uses reciprocal_approx_fast and is fused into
    the PSUM->SBUF attn-out copy;
  - xT is DMA'd in 4 column windows so projections start immediately.

All device compute is bf16 with fp32 PSUM accumulation. The host pre-casts
and pre-transposes x (xT) and pre-slices/reorders the weight shards so the
device performs no layout work on the inputs.
"""

import numpy as np

import concourse.bass as bass  # noqa: F401  (import keeps engine registry warm)
import concourse.mybir as mybir
import concourse.tile as tile
from concourse import bacc, bass_utils

# Problem shape (hardcoded per contract).
B, N, D = 2, 2048, 2048
NUM_HEADS = 32
NUM_KV_HEADS = 8
HD = 64                      # head dim
G = NUM_HEADS // NUM_KV_HEADS  # 4 query heads per kv head
N_CORES = 8
LQ = 8                       # local query heads per core (2 kv heads * G)
NT = D // 128                # 16 contraction tiles
NCHUNK = 4                   # token chunks of 512
CH = 512

_CACHE = {}


def _build():
    nc = bacc.Bacc("TRN2", target_bir_lowering=False, debug=False,
                   num_devices=N_CORES)
    f32, bf16 = mybir.dt.float32, mybir.dt.bfloat16

    xT = nc.dram_tensor("xT", [D, N], bf16, kind="ExternalInput")
    wq = nc.dram_tensor("wq", [D, 512], bf16, kind="ExternalInput")
    wk = nc.dram_tensor("wk", [D, 128], bf16, kind="ExternalInput")
    wv = nc.dram_tensor("wv", [D, 128], bf16, kind="ExternalInput")
    wo = nc.dram_tensor("wo", [512, D], bf16, kind="ExternalInput")
    msk = nc.dram_tensor("msk", [128, 256], bf16, kind="ExternalInput")
    part = nc.dram_tensor("part", [N, D], bf16, kind="ExternalOutput")

    with tile.TileContext(nc) as tc:
        with (
            tc.tile_pool(name="const", bufs=1) as cpool,
            tc.tile_pool(name="proj", bufs=1) as ppool,
            tc.tile_pool(name="work", bufs=4) as wpool,
            tc.tile_pool(name="att", bufs=1) as apool,
            tc.tile_pool(name="stage", bufs=3) as spool,
            tc.tile_pool(name="ps_s", bufs=2, space="PSUM") as ps_s,
            tc.tile_pool(name="ps_av", bufs=1, space="PSUM") as ps_av,
            tc.tile_pool(name="ps_m", bufs=2, space="PSUM") as ps_m,
        ):
            # ---- load constants / inputs to SBUF -------------------------
            # xT in 4 column windows so the first projection matmuls can
            # start as soon as window 0 lands.
            xt = cpool.tile([128, NT * N], bf16, tag="xt")
            for j in range(4):
                nc.sync.dma_start(
                    xt[:].rearrange("p (t n) -> p t n", t=NT)[:, :, j * CH:(j + 1) * CH],
                    xT.ap().rearrange("(t p) n -> p t n", p=128)[:, :, j * CH:(j + 1) * CH])
            wk_t = cpool.tile([128, NT * 128], bf16, tag="wk")
            nc.sync.dma_start(
                wk_t[:].rearrange("p (t o) -> p t o", t=NT),
                wk.ap().rearrange("(t p) o -> p t o", p=128))
            wv_t = cpool.tile([128, NT * 128], bf16, tag="wv")
            nc.sync.dma_start(
                wv_t[:].rearrange("p (t o) -> p t o", t=NT),
                wv.ap().rearrange("(t p) o -> p t o", p=128))
            wq_t = cpool.tile([128, NT * 512], bf16, tag="wq")
            nc.sync.dma_start(
                wq_t[:].rearrange("p (t o) -> p t o", t=NT),
                wq.ap().rearrange("(t p) o -> p t o", p=128))
            wo_t = cpool.tile([128, 4 * D], bf16, tag="wo")
            nc.sync.dma_start(
                wo_t[:].rearrange("p (t o) -> p t o", t=4),
                wo.ap().rearrange("(t p) o -> p t o", p=128))
            msk_t = cpool.tile([128, 256], bf16, tag="msk")
            nc.sync.dma_start(msk_t[:], msk.ap()[:])
            ones64 = cpool.tile([1, 64], f32, tag="ones64")
            nc.vector.memset(ones64[:], 1.0)

            # ---- projections --------------------------------------------
            # kT2 [128 (2 kv heads x 64), N]
            kt2 = ppool.tile([128, N], bf16, tag="kt2")
            for j in range(N // CH):
                ps = ps_m.tile([128, CH], f32, tag="misc")
                for t in range(NT):
                    nc.tensor.matmul(
                        ps[:], wk_t[:, t * 128:(t + 1) * 128],
                        xt[:, t * N + j * CH: t * N + (j + 1) * CH],
                        start=(t == 0), stop=(t == NT - 1))
                nc.scalar.activation(kt2[:, j * CH:(j + 1) * CH], ps[:],
                                     mybir.ActivationFunctionType.Copy)
            # vT [128, N] then DMA-transpose to V3 [128, 16*130].
            # Per m-tile layout: 0:64 = V_h0, 64 = ones, 65:129 = V_h1,
            # 129 = ones (the ones columns produce the softmax denominators
            # in row 64 of the attn@V PSUM output).
            v3 = apool.tile([128, 16 * 130], bf16, tag="v3")
            nc.vector.memset(v3[:], 1.0)
            for j in range(N // CH):
                ps = ps_m.tile([128, CH], f32, tag="misc")
                for t in range(NT):
                    nc.tensor.matmul(
                        ps[:], wv_t[:, t * 128:(t + 1) * 128],
                        xt[:, t * N + j * CH: t * N + (j + 1) * CH],
                        start=(t == 0), stop=(t == NT - 1))
                vt_s = spool.tile([128, CH], bf16, tag="vt")
                nc.scalar.activation(vt_s[:], ps[:],
                                     mybir.ActivationFunctionType.Copy)
                for s in range(4):       # 4 m-tiles of 128 in this chunk
                    mt = 4 * j + s
                    # full-tile DMA transpose (partition-offset inputs are
                    # broken in the xbar path), then split around the ones
                    # column with two free-dim-offset DVE copies.
                    vtr = spool.tile([128, 128], bf16, tag="vtr")
                    nc.sync.dma_start_transpose(
                        vtr[:], vt_s[:, s * 128:(s + 1) * 128])
                    nc.vector.tensor_copy(v3[:, mt * 130: mt * 130 + 64],
                                          vtr[:, 0:64])
                    nc.vector.tensor_copy(v3[:, mt * 130 + 65: mt * 130 + 129],
                                          vtr[:, 64:128])
            # qT2 chunks a=0..3: [128 (head a | head a+4), N]
            qt2 = []
            for a in range(4):
                qa = ppool.tile([128, N], bf16, tag=f"qt2_{a}")
                for j in range(N // CH):
                    ps = ps_m.tile([128, CH], f32, tag="misc")
                    for t in range(NT):
                        nc.tensor.matmul(
                            ps[:], wq_t[:, t * 512 + a * 128: t * 512 + (a + 1) * 128],
                            xt[:, t * N + j * CH: t * N + (j + 1) * CH],
                            start=(t == 0), stop=(t == NT - 1))
                    nc.scalar.activation(qa[:, j * CH:(j + 1) * CH], ps[:],
                                         mybir.ActivationFunctionType.Copy)
                qt2.append(qa)

            # ---- attention + o_proj per token chunk ---------------------
            for ci in range(NCHUNK):
                n0 = ci * CH
                mt_hi = 4 * ci + 4          # m-tiles 0..mt_hi-1
                aos = []                     # normalized attn outs per pair
                for a in range(4):
                    # pa: [0:64, 0:512]=head a+4 out, [0:64,512:1024]=head a
                    # out, row 64 of each half = softmax denominators.
                    # (h64 half goes in bank 0: walrus rejects tile_position
                    # row 64 combined with a non-zero PSUM output offset.)
                    pa = ps_av.tile([128, 2 * CH], f32, tag="av")
                    for mt in range(mt_hi):
                        diag = mt - 4 * ci
                        ss = ps_s.tile([128, 2 * CH], f32, tag="s")
                        # two 64-contraction matmuls on distinct PE row
                        # groups (h64 / h0), issued adjacently -> concurrent
                        nc.tensor.matmul(
                            ss[:, 0:CH], kt2[64:128, mt * 128:(mt + 1) * 128],
                            qt2[a][64:128, n0:n0 + CH],
                            start=True, stop=True)
                        nc.tensor.matmul(
                            ss[:, CH:2 * CH], kt2[0:64, mt * 128:(mt + 1) * 128],
                            qt2[a][0:64, n0:n0 + CH],
                            start=True, stop=True)
                        pt = wpool.tile([128, 2 * CH], bf16, tag="pt")
                        nc.scalar.activation(
                            pt[:], ss[:],
                            mybir.ActivationFunctionType.Exp, scale=0.125)
                        lo = 0
                        if diag >= 0:
                            # only the 128-wide block containing the diagonal
                            # needs the triangular mask; columns left of it
                            # are never streamed by the attn@V matmuls below.
                            lo = 128 * diag
                            ptv = pt[:].rearrange("p (h n) -> p h n", h=2)
                            mkv = msk_t[:].rearrange("p (h n) -> p h n", h=2)
                            nc.vector.tensor_mul(
                                ptv[:, :, lo:lo + 128], ptv[:, :, lo:lo + 128],
                                mkv)
                        nc.tensor.matmul(
                            pa[0:65, lo:CH],
                            v3[:, mt * 130 + 65: mt * 130 + 130],
                            pt[:, lo:CH],
                            start=(mt == 0), stop=(mt == mt_hi - 1),
                            skip_group_check=True)
                        nc.tensor.matmul(
                            pa[0:65, CH + lo:2 * CH],
                            v3[:, mt * 130: mt * 130 + 65],
                            pt[:, CH + lo:2 * CH],
                            start=(mt == 0), stop=(mt == mt_hi - 1),
                            skip_group_check=True)
                    # normalization: denominators -> broadcast -> approx
                    # reciprocal -> fused into the PSUM->SBUF copy.
                    sm = spool.tile([1, 2 * CH], f32, tag="sm")
                    nc.scalar.activation(sm[:], pa[64:65, :],
                                         mybir.ActivationFunctionType.Copy)
                    rb = ps_m.tile([128, CH], f32, tag="misc")
                    nc.tensor.matmul(rb[0:64, :], ones64[0:1, :],
                                     sm[0:1, CH:2 * CH],
                                     start=True, stop=True, tile_position=(0, 0))
                    nc.tensor.matmul(rb[64:128, :], ones64[0:1, :],
                                     sm[0:1, 0:CH],
                                     start=True, stop=True, tile_position=(0, 64))
                    rbr = spool.tile([128, CH], f32, tag="rbr")
                    nc.vector.reciprocal_approx_fast(rbr[:], rb[:])
                    an = apool.tile([128, CH], bf16, tag=f"aos_{a}")
                    nc.vector.tensor_mul(an[0:64, :], pa[0:64, CH:2 * CH],
                                         rbr[0:64, :])
                    nc.vector.tensor_mul(an[64:128, :], pa[0:64, 0:CH],
                                         rbr[64:128, :])
                    aos.append(an)
                # o_proj: out[n, :] += sum_c attn_outT_s[c, n] * Wo[c, :]
                for nt in range(4):
                    for dc in range(4):
                        po = ps_m.tile([128, CH], f32, tag="misc")
                        for a in range(4):
                            nc.tensor.matmul(
                                po[:], aos[a][:, nt * 128:(nt + 1) * 128],
                                wo_t[:, a * D + dc * CH: a * D + (dc + 1) * CH],
                                start=(a == 0), stop=(a == 3))
                        st = spool.tile([128, CH], bf16, tag="ost")
                        nc.vector.tensor_copy(st[:], po[:])
                        nc.sync.dma_start(
                            part.ap()[n0 + nt * 128: n0 + (nt + 1) * 128,
                                      dc * CH:(dc + 1) * CH],
                            st[:])
    nc.compile()
    return nc


def _prep_in_maps(x, Wq, Wk, Wv, Wo):
    import jax.numpy as jnp

    def to_bf16(a):
        return np.asarray(jnp.asarray(np.asarray(a), dtype=jnp.bfloat16))

    # 128x128 triangular causal mask, duplicated side by side so one DVE
    # multiply covers both head-halves of the diagonal block of a pt tile.
    i = np.arange(128)[:, None]
    j = np.arange(128)[None, :]
    tri = (i <= j).astype(np.float32)
    msk = np.concatenate([tri, tri], axis=1)

    in_maps = []
    for c in range(N_CORES):
        b, g = c // 4, c % 4
        qh = [8 * g + a for a in range(8)]      # global q heads for this core
        # Wq columns reordered into pair chunks [head a | head a+4]
        wq_cols = []
        for a in range(4):
            wq_cols.append(np.arange(qh[a] * HD, (qh[a] + 1) * HD))
            wq_cols.append(np.arange(qh[a + 4] * HD, (qh[a + 4] + 1) * HD))
        wq_r = np.asarray(Wq)[:, np.concatenate(wq_cols)]
        wo_rows = wq_cols  # same ordering for Wo rows
        wo_r = np.asarray(Wo)[np.concatenate(wo_rows), :]
        wk_s = np.asarray(Wk)[:, 2 * g * HD: (2 * g + 2) * HD]
        wv_s = np.asarray(Wv)[:, 2 * g * HD: (2 * g + 2) * HD]
        in_maps.append({
            "xT": to_bf16(np.asarray(x)[b].T),
            "wq": to_bf16(wq_r),
            "wk": to_bf16(wk_s),
            "wv": to_bf16(wv_s),
            "wo": to_bf16(wo_r),
            "msk": to_bf16(msk),
        })
    return in_maps


def kernel(x, Wq, Wk, Wv, Wo, trace=False):
    if "nc" not in _CACHE:
        _CACHE["nc"] = _build()
    nc = _CACHE["nc"]
    in_maps = _prep_in_maps(x, Wq, Wk, Wv, Wo)
    res = bass_utils.run_bass_kernel_spmd(
        nc, in_maps, core_ids=list(range(N_CORES)), trace=trace)
    _CACHE["last_result"] = res
    out = np.zeros((B, N, D), np.float32)
    for c in range(N_CORES):
        out[c // 4] += np.asarray(res.results[c]["part"], dtype=np.float32)
    return out


# revision 25
# speedup vs baseline: 1.6578x; 1.0168x over previous
"""GroupedQueryAttention forward on 8 Trainium2 NeuronCores (Bass/Tile).

Sharding (per spec hint): data-parallel over batch (B=2) x tensor-parallel
over KV-head groups (4 groups of 2 KV heads + their 8 query heads each).
Core c -> (batch b = c // 4, group g = c % 4).

Each core computes, for its batch element and its 8 query heads:
  qT/kT projections in transposed layout (lhsT = W, rhs = xT), V natural via
  on-chip PE transpose of vT; causal softmax without max-subtraction (scores
  are ~N(0,1) after the 1/sqrt(hd) scale, exp cannot overflow); the softmax
  denominator is produced by the same matmul as attn@V via a ones-column
  appended to V.
  o_proj is row-parallel: each core emits a full [N, D] fp32 partial, and the
  host sums the 4 partials per batch element (the "all-reduce" of the o_proj).

Perf structure vs the v1 kernel:
  - the two 64-contraction score matmuls of a head pair write one 2-bank
    PSUM tile [128,1024] and are issued adjacently so the PE row-group
    tiling (h0 rows 0-63 / h64 rows 64-127) runs them concurrently;
  - exp runs once per (pair, mt) over [128,1024] (halves ACT instruction
    overhead), mask is one [128,1024] DVE multiply;
  - softmax normalization uses reciprocal_approx_fast and is fused into
    the PSUM->SBUF attn-out copy;
  - xT is DMA'd in 4 column windows so projections start immediately.

All device compute is bf16 with fp32 PSUM accumulation. The host pre-casts
and pre-transposes x (xT) and pre-slices/reorders the weight shards so the
device performs no layout work on the inputs.
"""

import numpy as np

import concourse.bass as bass  # noqa: F401  (import keeps engine registry warm)
import concourse.mybir as mybir
import concourse.tile as tile
from concourse import bacc, bass_utils

# Problem shape (hardcoded per contract).
B, N, D = 2, 2048, 2048
NUM_HEADS = 32
NUM_KV_HEADS = 8
HD = 64                      # head dim
G = NUM_HEADS // NUM_KV_HEADS  # 4 query heads per kv head
N_CORES = 8
LQ = 8                       # local query heads per core (2 kv heads * G)
NT = D // 128                # 16 contraction tiles
NCHUNK = 4                   # token chunks of 512
CH = 512

_CACHE = {}


def _build():
    nc = bacc.Bacc("TRN2", target_bir_lowering=False, debug=False,
                   num_devices=N_CORES)
    f32, bf16 = mybir.dt.float32, mybir.dt.bfloat16

    xT = nc.dram_tensor("xT", [D, N], bf16, kind="ExternalInput")
    wq = nc.dram_tensor("wq", [D, 512], bf16, kind="ExternalInput")
    wk = nc.dram_tensor("wk", [D, 128], bf16, kind="ExternalInput")
    wv = nc.dram_tensor("wv", [D, 128], bf16, kind="ExternalInput")
    wo = nc.dram_tensor("wo", [512, D], bf16, kind="ExternalInput")
    msk = nc.dram_tensor("msk", [128, 256], bf16, kind="ExternalInput")
    part = nc.dram_tensor("part", [N, D], bf16, kind="ExternalOutput")

    with tile.TileContext(nc) as tc:
        with (
            tc.tile_pool(name="const", bufs=1) as cpool,
            tc.tile_pool(name="proj", bufs=1) as ppool,
            tc.tile_pool(name="work", bufs=4) as wpool,
            tc.tile_pool(name="att", bufs=1) as apool,
            tc.tile_pool(name="stage", bufs=3) as spool,
            tc.tile_pool(name="ps_s", bufs=2, space="PSUM") as ps_s,
            tc.tile_pool(name="ps_av", bufs=1, space="PSUM") as ps_av,
            tc.tile_pool(name="ps_m", bufs=2, space="PSUM") as ps_m,
        ):
            # ---- load constants / inputs to SBUF -------------------------
            # Input DMA is HBM-bandwidth bound (~37us for all inputs), so
            # order by first use: K/V weights, first xT token-window (so the
            # K projection starts ~8us in), Wq, remaining xT windows, Wo.
            wk_t = cpool.tile([128, NT * 128], bf16, tag="wk")
            nc.sync.dma_start(
                wk_t[:].rearrange("p (t o) -> p t o", t=NT),
                wk.ap().rearrange("(t p) o -> p t o", p=128))
            wv_t = cpool.tile([128, NT * 128], bf16, tag="wv")
            nc.sync.dma_start(
                wv_t[:].rearrange("p (t o) -> p t o", t=NT),
                wv.ap().rearrange("(t p) o -> p t o", p=128))
            msk_t = cpool.tile([128, 256], bf16, tag="msk")
            nc.sync.dma_start(msk_t[:], msk.ap()[:])
            xt = cpool.tile([128, NT * N], bf16, tag="xt")
            xtv = xt[:].rearrange("p (t n) -> p t n", t=NT)
            xsv = xT.ap().rearrange("(t p) n -> p t n", p=128)
            nc.sync.dma_start(xtv[:, :, 0:CH], xsv[:, :, 0:CH])
            wq_t = cpool.tile([128, NT * 512], bf16, tag="wq")
            nc.sync.dma_start(
                wq_t[:].rearrange("p (t o) -> p t o", t=NT),
                wq.ap().rearrange("(t p) o -> p t o", p=128))
            for j in range(1, 4):
                nc.sync.dma_start(
                    xtv[:, :, j * CH:(j + 1) * CH], xsv[:, :, j * CH:(j + 1) * CH])
            wo_t = cpool.tile([128, 4 * D], bf16, tag="wo")
            nc.sync.dma_start(
                wo_t[:].rearrange("p (t o) -> p t o", t=4),
                wo.ap().rearrange("(t p) o -> p t o", p=128))
            ones64 = cpool.tile([1, 64], f32, tag="ones64")
            nc.vector.memset(ones64[:], 1.0)

            # ---- projections --------------------------------------------
            # kT2 [128 (2 kv heads x 64), N]
            kt2 = ppool.tile([128, N], bf16, tag="kt2")
            for j in range(N // CH):
                ps = ps_m.tile([128, CH], f32, tag="misc")
                for t in range(NT):
                    nc.tensor.matmul(
                        ps[:], wk_t[:, t * 128:(t + 1) * 128],
                        xt[:, t * N + j * CH: t * N + (j + 1) * CH],
                        start=(t == 0), stop=(t == NT - 1))
                nc.scalar.activation(kt2[:, j * CH:(j + 1) * CH], ps[:],
                                     mybir.ActivationFunctionType.Copy)
            # vT [128, N] then DMA-transpose to V3 [128, 16*130].
            # Per m-tile layout: 0:64 = V_h0, 64 = ones, 65:129 = V_h1,
            # 129 = ones (the ones columns produce the softmax denominators
            # in row 64 of the attn@V PSUM output).
            v3 = apool.tile([128, 16 * 130], bf16, tag="v3")
            nc.vector.memset(v3[:], 1.0)
            for j in range(N // CH):
                ps = ps_m.tile([128, CH], f32, tag="misc")
                for t in range(NT):
                    nc.tensor.matmul(
                        ps[:], wv_t[:, t * 128:(t + 1) * 128],
                        xt[:, t * N + j * CH: t * N + (j + 1) * CH],
                        start=(t == 0), stop=(t == NT - 1))
                vt_s = spool.tile([128, CH], bf16, tag="vt")
                nc.scalar.activation(vt_s[:], ps[:],
                                     mybir.ActivationFunctionType.Copy)
                for s in range(4):       # 4 m-tiles of 128 in this chunk
                    mt = 4 * j + s
                    # full-tile DMA transpose (partition-offset inputs are
                    # broken in the xbar path), then split around the ones
                    # column with two free-dim-offset DVE copies.
                    vtr = spool.tile([128, 128], bf16, tag="vtr")
                    nc.sync.dma_start_transpose(
                        vtr[:], vt_s[:, s * 128:(s + 1) * 128])
                    nc.vector.tensor_copy(v3[:, mt * 130: mt * 130 + 64],
                                          vtr[:, 0:64])
                    nc.vector.tensor_copy(v3[:, mt * 130 + 65: mt * 130 + 129],
                                          vtr[:, 64:128])
            # qT2 chunks a=0..3: [128 (head a | head a+4), N]
            qt2 = []
            for a in range(4):
                qa = ppool.tile([128, N], bf16, tag=f"qt2_{a}")
                for j in range(N // CH):
                    ps = ps_m.tile([128, CH], f32, tag="misc")
                    for t in range(NT):
                        nc.tensor.matmul(
                            ps[:], wq_t[:, t * 512 + a * 128: t * 512 + (a + 1) * 128],
                            xt[:, t * N + j * CH: t * N + (j + 1) * CH],
                            start=(t == 0), stop=(t == NT - 1))
                    nc.scalar.activation(qa[:, j * CH:(j + 1) * CH], ps[:],
                                         mybir.ActivationFunctionType.Copy)
                qt2.append(qa)

            # ---- attention + o_proj per token chunk ---------------------
            for ci in range(NCHUNK):
                n0 = ci * CH
                mt_hi = 4 * ci + 4          # m-tiles 0..mt_hi-1
                aos = []                     # normalized attn outs per pair
                for a in range(4):
                    # pa: [0:64, 0:512]=head a+4 out, [0:64,512:1024]=head a
                    # out, row 64 of each half = softmax denominators.
                    # (h64 half goes in bank 0: walrus rejects tile_position
                    # row 64 combined with a non-zero PSUM output offset.)
                    pa = ps_av.tile([128, 2 * CH], f32, tag="av")
                    for mt in range(mt_hi):
                        diag = mt - 4 * ci
                        ss = ps_s.tile([128, 2 * CH], f32, tag="s")
                        # two 64-contraction matmuls on distinct PE row
                        # groups (h64 / h0), issued adjacently -> concurrent
                        nc.tensor.matmul(
                            ss[:, 0:CH], kt2[64:128, mt * 128:(mt + 1) * 128],
                            qt2[a][64:128, n0:n0 + CH],
                            start=True, stop=True)
                        nc.tensor.matmul(
                            ss[:, CH:2 * CH], kt2[0:64, mt * 128:(mt + 1) * 128],
                            qt2[a][0:64, n0:n0 + CH],
                            start=True, stop=True)
                        pt = wpool.tile([128, 2 * CH], bf16, tag="pt")
                        nc.scalar.activation(
                            pt[:], ss[:],
                            mybir.ActivationFunctionType.Exp, scale=0.125)
                        lo = 0
                        if diag >= 0:
                            # only the 128-wide block containing the diagonal
                            # needs the triangular mask; columns left of it
                            # are never streamed by the attn@V matmuls below.
                            lo = 128 * diag
                            ptv = pt[:].rearrange("p (h n) -> p h n", h=2)
                            mkv = msk_t[:].rearrange("p (h n) -> p h n", h=2)
                            nc.vector.tensor_mul(
                                ptv[:, :, lo:lo + 128], ptv[:, :, lo:lo + 128],
                                mkv)
                        nc.tensor.matmul(
                            pa[0:65, lo:CH],
                            v3[:, mt * 130 + 65: mt * 130 + 130],
                            pt[:, lo:CH],
                            start=(mt == 0), stop=(mt == mt_hi - 1),
                            skip_group_check=True)
                        nc.tensor.matmul(
                            pa[0:65, CH + lo:2 * CH],
                            v3[:, mt * 130: mt * 130 + 65],
                            pt[:, CH + lo:2 * CH],
                            start=(mt == 0), stop=(mt == mt_hi - 1),
                            skip_group_check=True)
                    # normalization: denominators -> broadcast -> approx
                    # reciprocal -> fused into the PSUM->SBUF copy.
                    sm = spool.tile([1, 2 * CH], f32, tag="sm")
                    nc.vector.tensor_copy(sm[:], pa[64:65, :])
                    rb = ps_m.tile([128, CH], f32, tag="misc")
                    nc.tensor.matmul(rb[0:64, :], ones64[0:1, :],
                                     sm[0:1, CH:2 * CH],
                                     start=True, stop=True, tile_position=(0, 0))
                    nc.tensor.matmul(rb[64:128, :], ones64[0:1, :],
                                     sm[0:1, 0:CH],
                                     start=True, stop=True, tile_position=(0, 64))
                    rbr = spool.tile([128, CH], f32, tag="rbr")
                    nc.vector.reciprocal_approx_fast(rbr[:], rb[:])
                    an = apool.tile([128, CH], bf16, tag=f"aos_{a}")
                    nc.vector.tensor_mul(an[0:64, :], pa[0:64, CH:2 * CH],
                                         rbr[0:64, :])
                    nc.vector.tensor_mul(an[64:128, :], pa[0:64, 0:CH],
                                         rbr[64:128, :])
                    aos.append(an)
                # o_proj: out[n, :] += sum_c attn_outT_s[c, n] * Wo[c, :]
                for nt in range(4):
                    for dc in range(4):
                        po = ps_m.tile([128, CH], f32, tag="misc")
                        for a in range(4):
                            nc.tensor.matmul(
                                po[:], aos[a][:, nt * 128:(nt + 1) * 128],
                                wo_t[:, a * D + dc * CH: a * D + (dc + 1) * CH],
                                start=(a == 0), stop=(a == 3))
                        st = spool.tile([128, CH], bf16, tag="ost")
                        nc.vector.tensor_copy(st[:], po[:])
                        nc.sync.dma_start(
                            part.ap()[n0 + nt * 128: n0 + (nt + 1) * 128,
                                      dc * CH:(dc + 1) * CH],
                            st[:])
    nc.compile()
    return nc


def _prep_in_maps(x, Wq, Wk, Wv, Wo):
    import jax.numpy as jnp

    def to_bf16(a):
        return np.asarray(jnp.asarray(np.asarray(a), dtype=jnp.bfloat16))

    # 128x128 triangular causal mask, duplicated side by side so one DVE
    # multiply covers both head-halves of the diagonal block of a pt tile.
    i = np.arange(128)[:, None]
    j = np.arange(128)[None, :]
    tri = (i <= j).astype(np.float32)
    msk = np.concatenate([tri, tri], axis=1)

    in_maps = []
    for c in range(N_CORES):
        b, g = c // 4, c % 4
        qh = [8 * g + a for a in range(8)]      # global q heads for this core
        # Wq columns reordered into pair chunks [head a | head a+4]
        wq_cols = []
        for a in range(4):
            wq_cols.append(np.arange(qh[a] * HD, (qh[a] + 1) * HD))
            wq_cols.append(np.arange(qh[a + 4] * HD, (qh[a + 4] + 1) * HD))
        wq_r = np.asarray(Wq)[:, np.concatenate(wq_cols)]
        wo_rows = wq_cols  # same ordering for Wo rows
        wo_r = np.asarray(Wo)[np.concatenate(wo_rows), :]
        wk_s = np.asarray(Wk)[:, 2 * g * HD: (2 * g + 2) * HD]
        wv_s = np.asarray(Wv)[:, 2 * g * HD: (2 * g + 2) * HD]
        in_maps.append({
            "xT": to_bf16(np.asarray(x)[b].T),
            "wq": to_bf16(wq_r),
            "wk": to_bf16(wk_s),
            "wv": to_bf16(wv_s),
            "wo": to_bf16(wo_r),
            "msk": to_bf16(msk),
        })
    return in_maps


def kernel(x, Wq, Wk, Wv, Wo, trace=False):
    if "nc" not in _CACHE:
        _CACHE["nc"] = _build()
    nc = _CACHE["nc"]
    in_maps = _prep_in_maps(x, Wq, Wk, Wv, Wo)
    res = bass_utils.run_bass_kernel_spmd(
        nc, in_maps, core_ids=list(range(N_CORES)), trace=trace)
    _CACHE["last_result"] = res
    out = np.zeros((B, N, D), np.float32)
    for c in range(N_CORES):
        out[c // 4] += np.asarray(res.results[c]["part"], dtype=np.float32)
    return out


# revision 27
# speedup vs baseline: 1.7119x; 1.0326x over previous
"""GroupedQueryAttention forward on 8 Trainium2 NeuronCores (Bass/Tile).

Sharding (per spec hint): data-parallel over batch (B=2) x tensor-parallel
over KV-head groups (4 groups of 2 KV heads + their 8 query heads each).
Core c -> (batch b = c // 4, group g = c % 4).

Each core computes, for its batch element and its 8 query heads:
  qT/kT projections in transposed layout (lhsT = W, rhs = xT), V natural via
  on-chip PE transpose of vT; causal softmax without max-subtraction (scores
  are ~N(0,1) after the 1/sqrt(hd) scale, exp cannot overflow); the softmax
  denominator is produced by the same matmul as attn@V via a ones-column
  appended to V.
  o_proj is row-parallel: each core emits a full [N, D] fp32 partial, and the
  host sums the 4 partials per batch element (the "all-reduce" of the o_proj).

Perf structure vs the v1 kernel:
  - the two 64-contraction score matmuls of a head pair write one 2-bank
    PSUM tile [128,1024] and are issued adjacently so the PE row-group
    tiling (h0 rows 0-63 / h64 rows 64-127) runs them concurrently;
  - exp runs once per (pair, mt) over [128,1024] (halves ACT instruction
    overhead), mask is one [128,1024] DVE multiply;
  - softmax normalization uses reciprocal_approx_fast and is fused into
    the PSUM->SBUF attn-out copy;
  - xT is DMA'd in 4 column windows so projections start immediately.

All device compute is bf16 with fp32 PSUM accumulation. The host pre-casts
and pre-transposes x (xT) and pre-slices/reorders the weight shards so the
device performs no layout work on the inputs.
"""

import numpy as np

import concourse.bass as bass  # noqa: F401  (import keeps engine registry warm)
import concourse.mybir as mybir
import concourse.tile as tile
from concourse import bacc, bass_utils

# Problem shape (hardcoded per contract).
B, N, D = 2, 2048, 2048
NUM_HEADS = 32
NUM_KV_HEADS = 8
HD = 64                      # head dim
G = NUM_HEADS // NUM_KV_HEADS  # 4 query heads per kv head
N_CORES = 8
LQ = 8                       # local query heads per core (2 kv heads * G)
NT = D // 128                # 16 contraction tiles
NCHUNK = 4                   # token chunks of 512
CH = 512

_CACHE = {}


def _build():
    nc = bacc.Bacc("TRN2", target_bir_lowering=False, debug=False,
                   num_devices=N_CORES)
    f32, bf16 = mybir.dt.float32, mybir.dt.bfloat16

    xT = nc.dram_tensor("xT", [D, N], bf16, kind="ExternalInput")
    wq = nc.dram_tensor("wq", [D, 512], bf16, kind="ExternalInput")
    wk = nc.dram_tensor("wk", [D, 128], bf16, kind="ExternalInput")
    wv = nc.dram_tensor("wv", [D, 128], bf16, kind="ExternalInput")
    wo = nc.dram_tensor("wo", [512, D], bf16, kind="ExternalInput")
    msk = nc.dram_tensor("msk", [128, 256], bf16, kind="ExternalInput")
    part = nc.dram_tensor("part", [N, D], bf16, kind="ExternalOutput")

    with tile.TileContext(nc) as tc:
        with (
            tc.tile_pool(name="const", bufs=1) as cpool,
            tc.tile_pool(name="proj", bufs=1) as ppool,
            tc.tile_pool(name="work", bufs=4) as wpool,
            tc.tile_pool(name="att", bufs=1) as apool,
            tc.tile_pool(name="stage", bufs=3) as spool,
            tc.tile_pool(name="ps_s", bufs=2, space="PSUM") as ps_s,
            tc.tile_pool(name="ps_av", bufs=1, space="PSUM") as ps_av,
            tc.tile_pool(name="ps_m", bufs=2, space="PSUM") as ps_m,
        ):
            # ---- load constants / inputs to SBUF -------------------------
            # Input DMA is HBM-bandwidth bound (~37us for all inputs), so
            # order by first use: K/V weights, first xT token-window (so the
            # K projection starts ~8us in), Wq, remaining xT windows, Wo.
            wk_t = cpool.tile([128, NT * 128], bf16, tag="wk")
            nc.sync.dma_start(
                wk_t[:].rearrange("p (t o) -> p t o", t=NT),
                wk.ap().rearrange("(t p) o -> p t o", p=128))
            wv_t = cpool.tile([128, NT * 128], bf16, tag="wv")
            nc.sync.dma_start(
                wv_t[:].rearrange("p (t o) -> p t o", t=NT),
                wv.ap().rearrange("(t p) o -> p t o", p=128))
            msk_t = cpool.tile([128, 256], bf16, tag="msk")
            nc.sync.dma_start(msk_t[:], msk.ap()[:])
            xt = cpool.tile([128, NT * N], bf16, tag="xt")
            xtv = xt[:].rearrange("p (t n) -> p t n", t=NT)
            xsv = xT.ap().rearrange("(t p) n -> p t n", p=128)
            nc.sync.dma_start(xtv[:, :, 0:CH], xsv[:, :, 0:CH])
            wq_t = cpool.tile([128, NT * 512], bf16, tag="wq")
            nc.sync.dma_start(
                wq_t[:].rearrange("p (t o) -> p t o", t=NT),
                wq.ap().rearrange("(t p) o -> p t o", p=128))
            for j in range(1, 4):
                nc.sync.dma_start(
                    xtv[:, :, j * CH:(j + 1) * CH], xsv[:, :, j * CH:(j + 1) * CH])
            wo_t = cpool.tile([128, 4 * D], bf16, tag="wo")
            nc.sync.dma_start(
                wo_t[:].rearrange("p (t o) -> p t o", t=4),
                wo.ap().rearrange("(t p) o -> p t o", p=128))
            ones64 = cpool.tile([1, 64], f32, tag="ones64")
            nc.vector.memset(ones64[:], 1.0)

            # ---- projections --------------------------------------------
            # Emission is grouped by token-window j so the in-order PE queue
            # never blocks on a not-yet-DMA'd xT window while ready work for
            # an earlier window sits behind it.
            # kT2 [128 (2 kv heads x 64), N]
            kt2 = ppool.tile([128, N], bf16, tag="kt2")
            # v3: vT DMA-transposed per m-tile: 0:64 = V_h0, 64 = ones,
            # 65:129 = V_h1, 129 = ones (the ones columns produce the softmax
            # denominators in row 64 of the attn@V PSUM output).
            v3 = apool.tile([128, 16 * 130], bf16, tag="v3")
            nc.vector.memset(v3[:], 1.0)
            qt2 = []
            for a in range(4):
                qa = ppool.tile([128, N], bf16, tag=f"qt2_{a}")
                qt2.append(qa)
            for j in range(N // CH):
                ps = ps_m.tile([128, CH], f32, tag="misc")
                for t in range(NT):
                    nc.tensor.matmul(
                        ps[:], wk_t[:, t * 128:(t + 1) * 128],
                        xt[:, t * N + j * CH: t * N + (j + 1) * CH],
                        start=(t == 0), stop=(t == NT - 1))
                nc.scalar.activation(kt2[:, j * CH:(j + 1) * CH], ps[:],
                                     mybir.ActivationFunctionType.Copy)
                ps = ps_m.tile([128, CH], f32, tag="misc")
                for t in range(NT):
                    nc.tensor.matmul(
                        ps[:], wv_t[:, t * 128:(t + 1) * 128],
                        xt[:, t * N + j * CH: t * N + (j + 1) * CH],
                        start=(t == 0), stop=(t == NT - 1))
                vt_s = spool.tile([128, CH], bf16, tag="vt")
                nc.scalar.activation(vt_s[:], ps[:],
                                     mybir.ActivationFunctionType.Copy)
                for s in range(4):       # 4 m-tiles of 128 in this chunk
                    mt = 4 * j + s
                    # full-tile DMA transpose (partition-offset inputs are
                    # broken in the xbar path), then split around the ones
                    # column with two free-dim-offset DVE copies.
                    vtr = spool.tile([128, 128], bf16, tag="vtr")
                    nc.sync.dma_start_transpose(
                        vtr[:], vt_s[:, s * 128:(s + 1) * 128])
                    nc.vector.tensor_copy(v3[:, mt * 130: mt * 130 + 64],
                                          vtr[:, 0:64])
                    nc.vector.tensor_copy(v3[:, mt * 130 + 65: mt * 130 + 129],
                                          vtr[:, 64:128])
                # qT2 chunks a=0..3: [128 (head a | head a+4), N]
                for a in range(4):
                    ps = ps_m.tile([128, CH], f32, tag="misc")
                    for t in range(NT):
                        nc.tensor.matmul(
                            ps[:], wq_t[:, t * 512 + a * 128: t * 512 + (a + 1) * 128],
                            xt[:, t * N + j * CH: t * N + (j + 1) * CH],
                            start=(t == 0), stop=(t == NT - 1))
                    nc.scalar.activation(qt2[a][:, j * CH:(j + 1) * CH], ps[:],
                                         mybir.ActivationFunctionType.Copy)

            # ---- attention + o_proj per token chunk ---------------------
            for ci in range(NCHUNK):
                n0 = ci * CH
                mt_hi = 4 * ci + 4          # m-tiles 0..mt_hi-1
                aos = []                     # normalized attn outs per pair
                for a in range(4):
                    # pa: [0:64, 0:512]=head a+4 out, [0:64,512:1024]=head a
                    # out, row 64 of each half = softmax denominators.
                    # (h64 half goes in bank 0: walrus rejects tile_position
                    # row 64 combined with a non-zero PSUM output offset.)
                    pa = ps_av.tile([128, 2 * CH], f32, tag="av")
                    for mt in range(mt_hi):
                        diag = mt - 4 * ci
                        ss = ps_s.tile([128, 2 * CH], f32, tag="s")
                        # two 64-contraction matmuls on distinct PE row
                        # groups (h64 / h0), issued adjacently -> concurrent
                        nc.tensor.matmul(
                            ss[:, 0:CH], kt2[64:128, mt * 128:(mt + 1) * 128],
                            qt2[a][64:128, n0:n0 + CH],
                            start=True, stop=True)
                        nc.tensor.matmul(
                            ss[:, CH:2 * CH], kt2[0:64, mt * 128:(mt + 1) * 128],
                            qt2[a][0:64, n0:n0 + CH],
                            start=True, stop=True)
                        pt = wpool.tile([128, 2 * CH], bf16, tag="pt")
                        nc.scalar.activation(
                            pt[:], ss[:],
                            mybir.ActivationFunctionType.Exp, scale=0.125)
                        lo = 0
                        if diag >= 0:
                            # only the 128-wide block containing the diagonal
                            # needs the triangular mask; columns left of it
                            # are never streamed by the attn@V matmuls below.
                            lo = 128 * diag
                            ptv = pt[:].rearrange("p (h n) -> p h n", h=2)
                            mkv = msk_t[:].rearrange("p (h n) -> p h n", h=2)
                            nc.vector.tensor_mul(
                                ptv[:, :, lo:lo + 128], ptv[:, :, lo:lo + 128],
                                mkv)
                        nc.tensor.matmul(
                            pa[0:65, lo:CH],
                            v3[:, mt * 130 + 65: mt * 130 + 130],
                            pt[:, lo:CH],
                            start=(mt == 0), stop=(mt == mt_hi - 1),
                            skip_group_check=True)
                        nc.tensor.matmul(
                            pa[0:65, CH + lo:2 * CH],
                            v3[:, mt * 130: mt * 130 + 65],
                            pt[:, CH + lo:2 * CH],
                            start=(mt == 0), stop=(mt == mt_hi - 1),
                            skip_group_check=True)
                    # normalization: denominators -> broadcast -> approx
                    # reciprocal -> fused into the PSUM->SBUF copy.
                    sm = spool.tile([1, 2 * CH], f32, tag="sm")
                    nc.vector.tensor_copy(sm[:], pa[64:65, :])
                    rb = ps_m.tile([128, CH], f32, tag="misc")
                    nc.tensor.matmul(rb[0:64, :], ones64[0:1, :],
                                     sm[0:1, CH:2 * CH],
                                     start=True, stop=True, tile_position=(0, 0))
                    nc.tensor.matmul(rb[64:128, :], ones64[0:1, :],
                                     sm[0:1, 0:CH],
                                     start=True, stop=True, tile_position=(0, 64))
                    rbr = spool.tile([128, CH], f32, tag="rbr")
                    nc.vector.reciprocal_approx_fast(rbr[:], rb[:])
                    an = apool.tile([128, CH], bf16, tag=f"aos_{a}")
                    nc.vector.tensor_mul(an[0:64, :], pa[0:64, CH:2 * CH],
                                         rbr[0:64, :])
                    nc.vector.tensor_mul(an[64:128, :], pa[0:64, 0:CH],
                                         rbr[64:128, :])
                    aos.append(an)
                # o_proj: out[n, :] += sum_c attn_outT_s[c, n] * Wo[c, :]
                for nt in range(4):
                    for dc in range(4):
                        po = ps_m.tile([128, CH], f32, tag="misc")
                        for a in range(4):
                            nc.tensor.matmul(
                                po[:], aos[a][:, nt * 128:(nt + 1) * 128],
                                wo_t[:, a * D + dc * CH: a * D + (dc + 1) * CH],
                                start=(a == 0), stop=(a == 3))
                        st = spool.tile([128, CH], bf16, tag="ost")
                        nc.vector.tensor_copy(st[:], po[:])
                        nc.sync.dma_start(
                            part.ap()[n0 + nt * 128: n0 + (nt + 1) * 128,
                                      dc * CH:(dc + 1) * CH],
                            st[:])
    nc.compile()
    return nc


def _prep_in_maps(x, Wq, Wk, Wv, Wo):
    import jax.numpy as jnp

    def to_bf16(a):
        return np.asarray(jnp.asarray(np.asarray(a), dtype=jnp.bfloat16))

    # 128x128 triangular causal mask, duplicated side by side so one DVE
    # multiply covers both head-halves of the diagonal block of a pt tile.
    i = np.arange(128)[:, None]
    j = np.arange(128)[None, :]
    tri = (i <= j).astype(np.float32)
    msk = np.concatenate([tri, tri], axis=1)

    in_maps = []
    for c in range(N_CORES):
        b, g = c // 4, c % 4
        qh = [8 * g + a for a in range(8)]      # global q heads for this core
        # Wq columns reordered into pair chunks [head a | head a+4]
        wq_cols = []
        for a in range(4):
            wq_cols.append(np.arange(qh[a] * HD, (qh[a] + 1) * HD))
            wq_cols.append(np.arange(qh[a + 4] * HD, (qh[a + 4] + 1) * HD))
        wq_r = np.asarray(Wq)[:, np.concatenate(wq_cols)]
        wo_rows = wq_cols  # same ordering for Wo rows
        wo_r = np.asarray(Wo)[np.concatenate(wo_rows), :]
        wk_s = np.asarray(Wk)[:, 2 * g * HD: (2 * g + 2) * HD]
        wv_s = np.asarray(Wv)[:, 2 * g * HD: (2 * g + 2) * HD]
        in_maps.append({
            "xT": to_bf16(np.asarray(x)[b].T),
            "wq": to_bf16(wq_r),
            "wk": to_bf16(wk_s),
            "wv": to_bf16(wv_s),
            "wo": to_bf16(wo_r),
            "msk": to_bf16(msk),
        })
    return in_maps


def kernel(x, Wq, Wk, Wv, Wo, trace=False):
    if "nc" not in _CACHE:
        _CACHE["nc"] = _build()
    nc = _CACHE["nc"]
    in_maps = _prep_in_maps(x, Wq, Wk, Wv, Wo)
    res = bass_utils.run_bass_kernel_spmd(
        nc, in_maps, core_ids=list(range(N_CORES)), trace=trace)
    _CACHE["last_result"] = res
    out = np.zeros((B, N, D), np.float32)
    for c in range(N_CORES):
        out[c // 4] += np.asarray(res.results[c]["part"], dtype=np.float32)
    return out


# revision 32
# speedup vs baseline: 1.7654x; 1.0312x over previous
"""GroupedQueryAttention forward on 8 Trainium2 NeuronCores (Bass/Tile).

Sharding (per spec hint): data-parallel over batch (B=2) x tensor-parallel
over KV-head groups (4 groups of 2 KV heads + their 8 query heads each).
Core c -> (batch b = c // 4, group g = c % 4).

Each core computes, for its batch element and its 8 query heads:
  qT/kT projections in transposed layout (lhsT = W, rhs = xT), V natural via
  on-chip PE transpose of vT; causal softmax without max-subtraction (scores
  are ~N(0,1) after the 1/sqrt(hd) scale, exp cannot overflow); the softmax
  denominator is produced by the same matmul as attn@V via a ones-column
  appended to V.
  o_proj is row-parallel: each core emits a full [N, D] fp32 partial, and the
  host sums the 4 partials per batch element (the "all-reduce" of the o_proj).

Perf structure vs the v1 kernel:
  - the two 64-contraction score matmuls of a head pair write one 2-bank
    PSUM tile [128,1024] and are issued adjacently so the PE row-group
    tiling (h0 rows 0-63 / h64 rows 64-127) runs them concurrently;
  - exp runs once per (pair, mt) over [128,1024] (halves ACT instruction
    overhead), mask is one [128,1024] DVE multiply;
  - softmax normalization uses reciprocal_approx_fast and is fused into
    the PSUM->SBUF attn-out copy;
  - xT is DMA'd in 4 column windows so projections start immediately.

All device compute is bf16 with fp32 PSUM accumulation. The host pre-casts
and pre-transposes x (xT) and pre-slices/reorders the weight shards so the
device performs no layout work on the inputs.
"""

import numpy as np

import concourse.bass as bass  # noqa: F401  (import keeps engine registry warm)
import concourse.mybir as mybir
import concourse.tile as tile
from concourse import bacc, bass_utils

# Problem shape (hardcoded per contract).
B, N, D = 2, 2048, 2048
NUM_HEADS = 32
NUM_KV_HEADS = 8
HD = 64                      # head dim
G = NUM_HEADS // NUM_KV_HEADS  # 4 query heads per kv head
N_CORES = 8
LQ = 8                       # local query heads per core (2 kv heads * G)
NT = D // 128                # 16 contraction tiles
NCHUNK = 4                   # token chunks of 512
CH = 512

_CACHE = {}


def _build():
    nc = bacc.Bacc("TRN2", target_bir_lowering=False, debug=False,
                   num_devices=N_CORES)
    f32, bf16 = mybir.dt.float32, mybir.dt.bfloat16

    xT = nc.dram_tensor("xT", [D, N], bf16, kind="ExternalInput")
    wq = nc.dram_tensor("wq", [D, 512], bf16, kind="ExternalInput")
    wk = nc.dram_tensor("wk", [D, 128], bf16, kind="ExternalInput")
    wv = nc.dram_tensor("wv", [D, 128], bf16, kind="ExternalInput")
    wo = nc.dram_tensor("wo", [512, D], bf16, kind="ExternalInput")
    msk = nc.dram_tensor("msk", [128, 256], bf16, kind="ExternalInput")
    part = nc.dram_tensor("part", [N, D], bf16, kind="ExternalOutput")

    with tile.TileContext(nc) as tc:
        with (
            tc.tile_pool(name="const", bufs=1) as cpool,
            tc.tile_pool(name="proj", bufs=1) as ppool,
            tc.tile_pool(name="work", bufs=4) as wpool,
            tc.tile_pool(name="att", bufs=1) as apool,
            tc.tile_pool(name="stage", bufs=3) as spool,
            tc.tile_pool(name="ps_s", bufs=2, space="PSUM") as ps_s,
            tc.tile_pool(name="ps_av", bufs=1, space="PSUM") as ps_av,
            tc.tile_pool(name="ps_m", bufs=2, space="PSUM") as ps_m,
        ):
            # ---- load constants / inputs to SBUF -------------------------
            # Input DMA is HBM-bandwidth bound (~37us for all inputs), so
            # order by first use: K/V weights, first xT token-window (so the
            # K projection starts ~8us in), Wq, remaining xT windows, Wo.
            wk_t = cpool.tile([128, NT * 128], bf16, tag="wk")
            nc.sync.dma_start(
                wk_t[:].rearrange("p (t o) -> p t o", t=NT),
                wk.ap().rearrange("(t p) o -> p t o", p=128))
            wv_t = cpool.tile([128, NT * 128], bf16, tag="wv")
            nc.sync.dma_start(
                wv_t[:].rearrange("p (t o) -> p t o", t=NT),
                wv.ap().rearrange("(t p) o -> p t o", p=128))
            msk_t = cpool.tile([128, 256], bf16, tag="msk")
            nc.sync.dma_start(msk_t[:], msk.ap()[:])
            xt = cpool.tile([128, NT * N], bf16, tag="xt")
            xtv = xt[:].rearrange("p (t n) -> p t n", t=NT)
            xsv = xT.ap().rearrange("(t p) n -> p t n", p=128)
            nc.sync.dma_start(xtv[:, :, 0:CH], xsv[:, :, 0:CH])
            wq_t = cpool.tile([128, NT * 512], bf16, tag="wq")
            nc.sync.dma_start(
                wq_t[:].rearrange("p (t o) -> p t o", t=NT),
                wq.ap().rearrange("(t p) o -> p t o", p=128))
            for j in range(1, 4):
                nc.sync.dma_start(
                    xtv[:, :, j * CH:(j + 1) * CH], xsv[:, :, j * CH:(j + 1) * CH])
            wo_t = cpool.tile([128, 4 * D], bf16, tag="wo")
            nc.sync.dma_start(
                wo_t[:].rearrange("p (t o) -> p t o", t=4),
                wo.ap().rearrange("(t p) o -> p t o", p=128))
            ones64 = cpool.tile([1, 64], f32, tag="ones64")
            nc.vector.memset(ones64[:], 1.0)

            # ---- projections --------------------------------------------
            # Emission is grouped by token-window j so the in-order PE queue
            # never blocks on a not-yet-DMA'd xT window while ready work for
            # an earlier window sits behind it.
            # kT2 [128 (2 kv heads x 64), N]
            kt2 = ppool.tile([128, N], bf16, tag="kt2")
            # v3: vT DMA-transposed per m-tile: 0:64 = V_h0, 64 = ones,
            # 65:129 = V_h1, 129 = ones (the ones columns produce the softmax
            # denominators in row 64 of the attn@V PSUM output).
            v3 = apool.tile([128, 16 * 130], bf16, tag="v3")
            nc.vector.memset(v3[:], 1.0)
            qt2 = []
            for a in range(4):
                qa = ppool.tile([128, N], bf16, tag=f"qt2_{a}")
                qt2.append(qa)
            for j in range(N // CH):
                ps = ps_m.tile([128, CH], f32, tag="misc")
                for t in range(NT):
                    nc.tensor.matmul(
                        ps[:], wk_t[:, t * 128:(t + 1) * 128],
                        xt[:, t * N + j * CH: t * N + (j + 1) * CH],
                        start=(t == 0), stop=(t == NT - 1))
                nc.scalar.activation(kt2[:, j * CH:(j + 1) * CH], ps[:],
                                     mybir.ActivationFunctionType.Copy)
                ps = ps_m.tile([128, CH], f32, tag="misc")
                for t in range(NT):
                    nc.tensor.matmul(
                        ps[:], wv_t[:, t * 128:(t + 1) * 128],
                        xt[:, t * N + j * CH: t * N + (j + 1) * CH],
                        start=(t == 0), stop=(t == NT - 1))
                vt_s = spool.tile([128, CH], bf16, tag="vt")
                nc.scalar.activation(vt_s[:], ps[:],
                                     mybir.ActivationFunctionType.Copy)
                for s in range(4):       # 4 m-tiles of 128 in this chunk
                    mt = 4 * j + s
                    # full-tile DMA transpose (partition-offset inputs are
                    # broken in the xbar path), then split around the ones
                    # column with two free-dim-offset DVE copies.
                    vtr = spool.tile([128, 128], bf16, tag="vtr")
                    nc.sync.dma_start_transpose(
                        vtr[:], vt_s[:, s * 128:(s + 1) * 128])
                    nc.vector.tensor_copy(v3[:, mt * 130: mt * 130 + 64],
                                          vtr[:, 0:64])
                    nc.vector.tensor_copy(v3[:, mt * 130 + 65: mt * 130 + 129],
                                          vtr[:, 64:128])
                # qT2 chunks a=0..3: [128 (head a | head a+4), N]
                for a in range(4):
                    ps = ps_m.tile([128, CH], f32, tag="misc")
                    for t in range(NT):
                        nc.tensor.matmul(
                            ps[:], wq_t[:, t * 512 + a * 128: t * 512 + (a + 1) * 128],
                            xt[:, t * N + j * CH: t * N + (j + 1) * CH],
                            start=(t == 0), stop=(t == NT - 1))
                    nc.scalar.activation(qt2[a][:, j * CH:(j + 1) * CH], ps[:],
                                         mybir.ActivationFunctionType.Copy)

            # ---- attention + o_proj per token chunk ---------------------
            for ci in range(NCHUNK):
                n0 = ci * CH
                mt_hi = 4 * ci + 4          # m-tiles 0..mt_hi-1
                aos = []                     # normalized attn outs per pair
                for a in range(4):
                    # pa: [0:64, 0:512]=head a+4 out, [0:64,512:1024]=head a
                    # out, row 64 of each half = softmax denominators.
                    # (h64 half goes in bank 0: walrus rejects tile_position
                    # row 64 combined with a non-zero PSUM output offset.)
                    pa = ps_av.tile([128, 2 * CH], f32, tag="av")
                    for mt in range(mt_hi):
                        diag = mt - 4 * ci
                        # columns left of the diagonal 128-block are fully
                        # masked for every m -> skip them in the score
                        # matmuls, the exp, and the attn@V streams.
                        lo = 128 * diag if diag > 0 else 0
                        ss = ps_s.tile([128, 2 * CH], f32, tag="s")
                        # two 64-contraction matmuls on distinct PE row
                        # groups (h64 / h0), issued adjacently -> concurrent
                        nc.tensor.matmul(
                            ss[:, lo:CH], kt2[64:128, mt * 128:(mt + 1) * 128],
                            qt2[a][64:128, n0 + lo:n0 + CH],
                            start=True, stop=True)
                        nc.tensor.matmul(
                            ss[:, CH + lo:2 * CH],
                            kt2[0:64, mt * 128:(mt + 1) * 128],
                            qt2[a][0:64, n0 + lo:n0 + CH],
                            start=True, stop=True)
                        pt = wpool.tile([128, 2 * CH], bf16, tag="pt")
                        if lo:
                            ssv = ss[:].rearrange("p (h n) -> p h n", h=2)
                            ptv = pt[:].rearrange("p (h n) -> p h n", h=2)
                            nc.scalar.activation(
                                ptv[:, :, lo:], ssv[:, :, lo:],
                                mybir.ActivationFunctionType.Exp, scale=0.125)
                        else:
                            nc.scalar.activation(
                                pt[:], ss[:],
                                mybir.ActivationFunctionType.Exp, scale=0.125)
                        if diag >= 0:
                            # only the 128-wide block containing the diagonal
                            # needs the triangular mask
                            ptv = pt[:].rearrange("p (h n) -> p h n", h=2)
                            mkv = msk_t[:].rearrange("p (h n) -> p h n", h=2)
                            nc.vector.tensor_mul(
                                ptv[:, :, lo:lo + 128], ptv[:, :, lo:lo + 128],
                                mkv)
                        nc.tensor.matmul(
                            pa[0:65, lo:CH],
                            v3[:, mt * 130 + 65: mt * 130 + 130],
                            pt[:, lo:CH],
                            start=(mt == 0), stop=(mt == mt_hi - 1),
                            skip_group_check=True)
                        nc.tensor.matmul(
                            pa[0:65, CH + lo:2 * CH],
                            v3[:, mt * 130: mt * 130 + 65],
                            pt[:, CH + lo:2 * CH],
                            start=(mt == 0), stop=(mt == mt_hi - 1),
                            skip_group_check=True)
                    # normalization: copy attn-out + denominators to SBUF
                    # right away (frees the pa PSUM banks for the next
                    # pair), then broadcast -> approx reciprocal -> scale.
                    sm = spool.tile([1, 2 * CH], f32, tag="sm")
                    nc.vector.tensor_copy(sm[:], pa[64:65, :])
                    rb = ps_m.tile([128, CH], f32, tag="misc")
                    nc.tensor.matmul(rb[0:64, :], ones64[0:1, :],
                                     sm[0:1, CH:2 * CH],
                                     start=True, stop=True, tile_position=(0, 0))
                    nc.tensor.matmul(rb[64:128, :], ones64[0:1, :],
                                     sm[0:1, 0:CH],
                                     start=True, stop=True, tile_position=(0, 64))
                    rbr = spool.tile([128, CH], f32, tag="rbr")
                    nc.vector.reciprocal_approx_fast(rbr[:], rb[:])
                    an = apool.tile([128, CH], bf16, tag=f"aos_{a}")
                    nc.vector.tensor_mul(an[0:64, :], pa[0:64, CH:2 * CH],
                                         rbr[0:64, :])
                    nc.vector.tensor_mul(an[64:128, :], pa[0:64, 0:CH],
                                         rbr[64:128, :])
                    aos.append(an)
                # o_proj: out[n, :] += sum_c attn_outT_s[c, n] * Wo[c, :]
                for nt in range(4):
                    for dc in range(4):
                        po = ps_m.tile([128, CH], f32, tag="misc")
                        for a in range(4):
                            nc.tensor.matmul(
                                po[:], aos[a][:, nt * 128:(nt + 1) * 128],
                                wo_t[:, a * D + dc * CH: a * D + (dc + 1) * CH],
                                start=(a == 0), stop=(a == 3))
                        st = spool.tile([128, CH], bf16, tag="ost")
                        nc.vector.tensor_copy(st[:], po[:])
                        nc.sync.dma_start(
                            part.ap()[n0 + nt * 128: n0 + (nt + 1) * 128,
                                      dc * CH:(dc + 1) * CH],
                            st[:])
    nc.compile()
    return nc


def _prep_in_maps(x, Wq, Wk, Wv, Wo):
    import jax.numpy as jnp

    def to_bf16(a):
        return np.asarray(jnp.asarray(np.asarray(a), dtype=jnp.bfloat16))

    # 128x128 triangular causal mask, duplicated side by side so one DVE
    # multiply covers both head-halves of the diagonal block of a pt tile.
    i = np.arange(128)[:, None]
    j = np.arange(128)[None, :]
    tri = (i <= j).astype(np.float32)
    msk = np.concatenate([tri, tri], axis=1)

    in_maps = []
    for c in range(N_CORES):
        b, g = c // 4, c % 4
        qh = [8 * g + a for a in range(8)]      # global q heads for this core
        # Wq columns reordered into pair chunks [head a | head a+4]
        wq_cols = []
        for a in range(4):
            wq_cols.append(np.arange(qh[a] * HD, (qh[a] + 1) * HD))
            wq_cols.append(np.arange(qh[a + 4] * HD, (qh[a + 4] + 1) * HD))
        wq_r = np.asarray(Wq)[:, np.concatenate(wq_cols)]
        wo_rows = wq_cols  # same ordering for Wo rows
        wo_r = np.asarray(Wo)[np.concatenate(wo_rows), :]
        wk_s = np.asarray(Wk)[:, 2 * g * HD: (2 * g + 2) * HD]
        wv_s = np.asarray(Wv)[:, 2 * g * HD: (2 * g + 2) * HD]
        in_maps.append({
            "xT": to_bf16(np.asarray(x)[b].T),
            "wq": to_bf16(wq_r),
            "wk": to_bf16(wk_s),
            "wv": to_bf16(wv_s),
            "wo": to_bf16(wo_r),
            "msk": to_bf16(msk),
        })
    return in_maps


def kernel(x, Wq, Wk, Wv, Wo, trace=False):
    if "nc" not in _CACHE:
        _CACHE["nc"] = _build()
    nc = _CACHE["nc"]
    in_maps = _prep_in_maps(x, Wq, Wk, Wv, Wo)
    res = bass_utils.run_bass_kernel_spmd(
        nc, in_maps, core_ids=list(range(N_CORES)), trace=trace)
    _CACHE["last_result"] = res
    out = np.zeros((B, N, D), np.float32)
    for c in range(N_CORES):
        out[c // 4] += np.asarray(res.results[c]["part"], dtype=np.float32)
    return out


# revision 33
# speedup vs baseline: 1.7726x; 1.0041x over previous
"""GroupedQueryAttention forward on 8 Trainium2 NeuronCores (Bass/Tile).

Sharding (per spec hint): data-parallel over batch (B=2) x tensor-parallel
over KV-head groups (4 groups of 2 KV heads + their 8 query heads each).
Core c -> (batch b = c // 4, group g = c % 4).

Each core computes, for its batch element and its 8 query heads:
  qT/kT projections in transposed layout (lhsT = W, rhs = xT), V natural via
  on-chip PE transpose of vT; causal softmax without max-subtraction (scores
  are ~N(0,1) after the 1/sqrt(hd) scale, exp cannot overflow); the softmax
  denominator is produced by the same matmul as attn@V via a ones-column
  appended to V.
  o_proj is row-parallel: each core emits a full [N, D] fp32 partial, and the
  host sums the 4 partials per batch element (the "all-reduce" of the o_proj).

Perf structure vs the v1 kernel:
  - the two 64-contraction score matmuls of a head pair write one 2-bank
    PSUM tile [128,1024] and are issued adjacently so the PE row-group
    tiling (h0 rows 0-63 / h64 rows 64-127) runs them concurrently;
  - exp runs once per (pair, mt) over [128,1024] (halves ACT instruction
    overhead), mask is one [128,1024] DVE multiply;
  - softmax normalization uses reciprocal_approx_fast and is fused into
    the PSUM->SBUF attn-out copy;
  - xT is DMA'd in 4 column windows so projections start immediately.

All device compute is bf16 with fp32 PSUM accumulation. The host pre-casts
and pre-transposes x (xT) and pre-slices/reorders the weight shards so the
device performs no layout work on the inputs.
"""

import numpy as np

import concourse.bass as bass  # noqa: F401  (import keeps engine registry warm)
import concourse.mybir as mybir
import concourse.tile as tile
from concourse import bacc, bass_utils

# Problem shape (hardcoded per contract).
B, N, D = 2, 2048, 2048
NUM_HEADS = 32
NUM_KV_HEADS = 8
HD = 64                      # head dim
G = NUM_HEADS // NUM_KV_HEADS  # 4 query heads per kv head
N_CORES = 8
LQ = 8                       # local query heads per core (2 kv heads * G)
NT = D // 128                # 16 contraction tiles
NCHUNK = 4                   # token chunks of 512
CH = 512

_CACHE = {}


def _build():
    nc = bacc.Bacc("TRN2", target_bir_lowering=False, debug=False,
                   num_devices=N_CORES)
    f32, bf16 = mybir.dt.float32, mybir.dt.bfloat16

    xT = nc.dram_tensor("xT", [D, N], bf16, kind="ExternalInput")
    wq = nc.dram_tensor("wq", [D, 512], bf16, kind="ExternalInput")
    wk = nc.dram_tensor("wk", [D, 128], bf16, kind="ExternalInput")
    wv = nc.dram_tensor("wv", [D, 128], bf16, kind="ExternalInput")
    wo = nc.dram_tensor("wo", [512, D], bf16, kind="ExternalInput")
    msk = nc.dram_tensor("msk", [128, 256], bf16, kind="ExternalInput")
    part = nc.dram_tensor("part", [N, D], bf16, kind="ExternalOutput")

    with tile.TileContext(nc) as tc:
        with (
            tc.tile_pool(name="const", bufs=1) as cpool,
            tc.tile_pool(name="proj", bufs=1) as ppool,
            tc.tile_pool(name="work", bufs=4) as wpool,
            tc.tile_pool(name="att", bufs=1) as apool,
            tc.tile_pool(name="stage", bufs=3) as spool,
            tc.tile_pool(name="ps_s", bufs=2, space="PSUM") as ps_s,
            tc.tile_pool(name="ps_av", bufs=1, space="PSUM") as ps_av,
            tc.tile_pool(name="ps_m", bufs=2, space="PSUM") as ps_m,
        ):
            # ---- load constants / inputs to SBUF -------------------------
            # Input DMA is HBM-bandwidth bound (~37us for all inputs), so
            # order by first use: K/V weights, first xT token-window (so the
            # K projection starts ~8us in), Wq, remaining xT windows, Wo.
            xt = cpool.tile([128, NT * N], bf16, tag="xt")
            xtv = xt[:].rearrange("p (t n) -> p t n", t=NT)
            xsv = xT.ap().rearrange("(t p) n -> p t n", p=128)
            nc.sync.dma_start(xtv[:, :, 0:CH], xsv[:, :, 0:CH])
            wk_t = cpool.tile([128, NT * 128], bf16, tag="wk")
            nc.sync.dma_start(
                wk_t[:].rearrange("p (t o) -> p t o", t=NT),
                wk.ap().rearrange("(t p) o -> p t o", p=128))
            wv_t = cpool.tile([128, NT * 128], bf16, tag="wv")
            nc.sync.dma_start(
                wv_t[:].rearrange("p (t o) -> p t o", t=NT),
                wv.ap().rearrange("(t p) o -> p t o", p=128))
            msk_t = cpool.tile([128, 256], bf16, tag="msk")
            nc.sync.dma_start(msk_t[:], msk.ap()[:])
            wq_t = cpool.tile([128, NT * 512], bf16, tag="wq")
            nc.sync.dma_start(
                wq_t[:].rearrange("p (t o) -> p t o", t=NT),
                wq.ap().rearrange("(t p) o -> p t o", p=128))
            for j in range(1, 4):
                nc.sync.dma_start(
                    xtv[:, :, j * CH:(j + 1) * CH], xsv[:, :, j * CH:(j + 1) * CH])
            wo_t = cpool.tile([128, 4 * D], bf16, tag="wo")
            nc.sync.dma_start(
                wo_t[:].rearrange("p (t o) -> p t o", t=4),
                wo.ap().rearrange("(t p) o -> p t o", p=128))
            ones64 = cpool.tile([1, 64], f32, tag="ones64")
            nc.vector.memset(ones64[:], 1.0)

            # ---- projections --------------------------------------------
            # Emission is grouped by token-window j so the in-order PE queue
            # never blocks on a not-yet-DMA'd xT window while ready work for
            # an earlier window sits behind it.
            # kT2 [128 (2 kv heads x 64), N]
            kt2 = ppool.tile([128, N], bf16, tag="kt2")
            # v3: vT DMA-transposed per m-tile: 0:64 = V_h0, 64 = ones,
            # 65:129 = V_h1, 129 = ones (the ones columns produce the softmax
            # denominators in row 64 of the attn@V PSUM output).
            v3 = apool.tile([128, 16 * 130], bf16, tag="v3")
            nc.vector.memset(v3[:], 1.0)
            qt2 = []
            for a in range(4):
                qa = ppool.tile([128, N], bf16, tag=f"qt2_{a}")
                qt2.append(qa)
            for j in range(N // CH):
                ps = ps_m.tile([128, CH], f32, tag="misc")
                for t in range(NT):
                    nc.tensor.matmul(
                        ps[:], wk_t[:, t * 128:(t + 1) * 128],
                        xt[:, t * N + j * CH: t * N + (j + 1) * CH],
                        start=(t == 0), stop=(t == NT - 1))
                nc.scalar.activation(kt2[:, j * CH:(j + 1) * CH], ps[:],
                                     mybir.ActivationFunctionType.Copy)
                ps = ps_m.tile([128, CH], f32, tag="misc")
                for t in range(NT):
                    nc.tensor.matmul(
                        ps[:], wv_t[:, t * 128:(t + 1) * 128],
                        xt[:, t * N + j * CH: t * N + (j + 1) * CH],
                        start=(t == 0), stop=(t == NT - 1))
                vt_s = spool.tile([128, CH], bf16, tag="vt")
                nc.scalar.activation(vt_s[:], ps[:],
                                     mybir.ActivationFunctionType.Copy)
                for s in range(4):       # 4 m-tiles of 128 in this chunk
                    mt = 4 * j + s
                    # full-tile DMA transpose (partition-offset inputs are
                    # broken in the xbar path), then split around the ones
                    # column with two free-dim-offset DVE copies.
                    vtr = spool.tile([128, 128], bf16, tag="vtr")
                    nc.sync.dma_start_transpose(
                        vtr[:], vt_s[:, s * 128:(s + 1) * 128])
                    nc.vector.tensor_copy(v3[:, mt * 130: mt * 130 + 64],
                                          vtr[:, 0:64])
                    nc.vector.tensor_copy(v3[:, mt * 130 + 65: mt * 130 + 129],
                                          vtr[:, 64:128])
                # qT2 chunks a=0..3: [128 (head a | head a+4), N]
                for a in range(4):
                    ps = ps_m.tile([128, CH], f32, tag="misc")
                    for t in range(NT):
                        nc.tensor.matmul(
                            ps[:], wq_t[:, t * 512 + a * 128: t * 512 + (a + 1) * 128],
                            xt[:, t * N + j * CH: t * N + (j + 1) * CH],
                            start=(t == 0), stop=(t == NT - 1))
                    nc.scalar.activation(qt2[a][:, j * CH:(j + 1) * CH], ps[:],
                                         mybir.ActivationFunctionType.Copy)

            # ---- attention + o_proj per token chunk ---------------------
            for ci in range(NCHUNK):
                n0 = ci * CH
                mt_hi = 4 * ci + 4          # m-tiles 0..mt_hi-1
                aos = []                     # normalized attn outs per pair
                for a in range(4):
                    # pa: [0:64, 0:512]=head a+4 out, [0:64,512:1024]=head a
                    # out, row 64 of each half = softmax denominators.
                    # (h64 half goes in bank 0: walrus rejects tile_position
                    # row 64 combined with a non-zero PSUM output offset.)
                    pa = ps_av.tile([128, 2 * CH], f32, tag="av")
                    for mt in range(mt_hi):
                        diag = mt - 4 * ci
                        # columns left of the diagonal 128-block are fully
                        # masked for every m -> skip them in the score
                        # matmuls, the exp, and the attn@V streams.
                        lo = 128 * diag if diag > 0 else 0
                        ss = ps_s.tile([128, 2 * CH], f32, tag="s")
                        # two 64-contraction matmuls on distinct PE row
                        # groups (h64 / h0), issued adjacently -> concurrent
                        nc.tensor.matmul(
                            ss[:, lo:CH], kt2[64:128, mt * 128:(mt + 1) * 128],
                            qt2[a][64:128, n0 + lo:n0 + CH],
                            start=True, stop=True)
                        nc.tensor.matmul(
                            ss[:, CH + lo:2 * CH],
                            kt2[0:64, mt * 128:(mt + 1) * 128],
                            qt2[a][0:64, n0 + lo:n0 + CH],
                            start=True, stop=True)
                        pt = wpool.tile([128, 2 * CH], bf16, tag="pt")
                        if lo:
                            ssv = ss[:].rearrange("p (h n) -> p h n", h=2)
                            ptv = pt[:].rearrange("p (h n) -> p h n", h=2)
                            nc.scalar.activation(
                                ptv[:, :, lo:], ssv[:, :, lo:],
                                mybir.ActivationFunctionType.Exp, scale=0.125)
                        else:
                            nc.scalar.activation(
                                pt[:], ss[:],
                                mybir.ActivationFunctionType.Exp, scale=0.125)
                        if diag >= 0:
                            # only the 128-wide block containing the diagonal
                            # needs the triangular mask
                            ptv = pt[:].rearrange("p (h n) -> p h n", h=2)
                            mkv = msk_t[:].rearrange("p (h n) -> p h n", h=2)
                            nc.vector.tensor_mul(
                                ptv[:, :, lo:lo + 128], ptv[:, :, lo:lo + 128],
                                mkv)
                        nc.tensor.matmul(
                            pa[0:65, lo:CH],
                            v3[:, mt * 130 + 65: mt * 130 + 130],
                            pt[:, lo:CH],
                            start=(mt == 0), stop=(mt == mt_hi - 1),
                            skip_group_check=True)
                        nc.tensor.matmul(
                            pa[0:65, CH + lo:2 * CH],
                            v3[:, mt * 130: mt * 130 + 65],
                            pt[:, CH + lo:2 * CH],
                            start=(mt == 0), stop=(mt == mt_hi - 1),
                            skip_group_check=True)
                    # normalization: copy attn-out + denominators to SBUF
                    # right away (frees the pa PSUM banks for the next
                    # pair), then broadcast -> approx reciprocal -> scale.
                    sm = spool.tile([1, 2 * CH], f32, tag="sm")
                    nc.vector.tensor_copy(sm[:], pa[64:65, :])
                    rb = ps_m.tile([128, CH], f32, tag="misc")
                    nc.tensor.matmul(rb[0:64, :], ones64[0:1, :],
                                     sm[0:1, CH:2 * CH],
                                     start=True, stop=True, tile_position=(0, 0))
                    nc.tensor.matmul(rb[64:128, :], ones64[0:1, :],
                                     sm[0:1, 0:CH],
                                     start=True, stop=True, tile_position=(0, 64))
                    rbr = spool.tile([128, CH], f32, tag="rbr")
                    nc.vector.reciprocal_approx_fast(rbr[:], rb[:])
                    an = apool.tile([128, CH], bf16, tag=f"aos_{a}")
                    nc.vector.tensor_mul(an[0:64, :], pa[0:64, CH:2 * CH],
                                         rbr[0:64, :])
                    nc.vector.tensor_mul(an[64:128, :], pa[0:64, 0:CH],
                                         rbr[64:128, :])
                    aos.append(an)
                # o_proj: out[n, :] += sum_c attn_outT_s[c, n] * Wo[c, :]
                for nt in range(4):
                    for dc in range(4):
                        po = ps_m.tile([128, CH], f32, tag="misc")
                        for a in range(4):
                            nc.tensor.matmul(
                                po[:], aos[a][:, nt * 128:(nt + 1) * 128],
                                wo_t[:, a * D + dc * CH: a * D + (dc + 1) * CH],
                                start=(a == 0), stop=(a == 3))
                        st = spool.tile([128, CH], bf16, tag="ost")
                        nc.vector.tensor_copy(st[:], po[:])
                        nc.sync.dma_start(
                            part.ap()[n0 + nt * 128: n0 + (nt + 1) * 128,
                                      dc * CH:(dc + 1) * CH],
                            st[:])
    nc.compile()
    return nc


def _prep_in_maps(x, Wq, Wk, Wv, Wo):
    import jax.numpy as jnp

    def to_bf16(a):
        return np.asarray(jnp.asarray(np.asarray(a), dtype=jnp.bfloat16))

    # 128x128 triangular causal mask, duplicated side by side so one DVE
    # multiply covers both head-halves of the diagonal block of a pt tile.
    i = np.arange(128)[:, None]
    j = np.arange(128)[None, :]
    tri = (i <= j).astype(np.float32)
    msk = np.concatenate([tri, tri], axis=1)

    in_maps = []
    for c in range(N_CORES):
        b, g = c // 4, c % 4
        qh = [8 * g + a for a in range(8)]      # global q heads for this core
        # Wq columns reordered into pair chunks [head a | head a+4]
        wq_cols = []
        for a in range(4):
            wq_cols.append(np.arange(qh[a] * HD, (qh[a] + 1) * HD))
            wq_cols.append(np.arange(qh[a + 4] * HD, (qh[a + 4] + 1) * HD))
        wq_r = np.asarray(Wq)[:, np.concatenate(wq_cols)]
        wo_rows = wq_cols  # same ordering for Wo rows
        wo_r = np.asarray(Wo)[np.concatenate(wo_rows), :]
        wk_s = np.asarray(Wk)[:, 2 * g * HD: (2 * g + 2) * HD]
        wv_s = np.asarray(Wv)[:, 2 * g * HD: (2 * g + 2) * HD]
        in_maps.append({
            "xT": to_bf16(np.asarray(x)[b].T),
            "wq": to_bf16(wq_r),
            "wk": to_bf16(wk_s),
            "wv": to_bf16(wv_s),
            "wo": to_bf16(wo_r),
            "msk": to_bf16(msk),
        })
    return in_maps


def kernel(x, Wq, Wk, Wv, Wo, trace=False):
    if "nc" not in _CACHE:
        _CACHE["nc"] = _build()
    nc = _CACHE["nc"]
    in_maps = _prep_in_maps(x, Wq, Wk, Wv, Wo)
    res = bass_utils.run_bass_kernel_spmd(
        nc, in_maps, core_ids=list(range(N_CORES)), trace=trace)
    _CACHE["last_result"] = res
    out = np.zeros((B, N, D), np.float32)
    for c in range(N_CORES):
        out[c // 4] += np.asarray(res.results[c]["part"], dtype=np.float32)
    return out


# revision 34
# speedup vs baseline: 1.7835x; 1.0061x over previous
"""GroupedQueryAttention forward on 8 Trainium2 NeuronCores (Bass/Tile).

Sharding (per spec hint): data-parallel over batch (B=2) x tensor-parallel
over KV-head groups (4 groups of 2 KV heads + their 8 query heads each).
Core c -> (batch b = c // 4, group g = c % 4).

Each core computes, for its batch element and its 8 query heads:
  qT/kT projections in transposed layout (lhsT = W, rhs = xT), V natural via
  on-chip PE transpose of vT; causal softmax without max-subtraction (scores
  are ~N(0,1) after the 1/sqrt(hd) scale, exp cannot overflow); the softmax
  denominator is produced by the same matmul as attn@V via a ones-column
  appended to V.
  o_proj is row-parallel: each core emits a full [N, D] fp32 partial, and the
  host sums the 4 partials per batch element (the "all-reduce" of the o_proj).

Perf structure vs the v1 kernel:
  - the two 64-contraction score matmuls of a head pair write one 2-bank
    PSUM tile [128,1024] and are issued adjacently so the PE row-group
    tiling (h0 rows 0-63 / h64 rows 64-127) runs them concurrently;
  - exp runs once per (pair, mt) over [128,1024] (halves ACT instruction
    overhead), mask is one [128,1024] DVE multiply;
  - softmax normalization uses reciprocal_approx_fast and is fused into
    the PSUM->SBUF attn-out copy;
  - xT is DMA'd in 4 column windows so projections start immediately.

All device compute is bf16 with fp32 PSUM accumulation. The host pre-casts
and pre-transposes x (xT) and pre-slices/reorders the weight shards so the
device performs no layout work on the inputs.
"""

import numpy as np

import concourse.bass as bass  # noqa: F401  (import keeps engine registry warm)
import concourse.mybir as mybir
import concourse.tile as tile
from concourse import bacc, bass_utils

# Problem shape (hardcoded per contract).
B, N, D = 2, 2048, 2048
NUM_HEADS = 32
NUM_KV_HEADS = 8
HD = 64                      # head dim
G = NUM_HEADS // NUM_KV_HEADS  # 4 query heads per kv head
N_CORES = 8
LQ = 8                       # local query heads per core (2 kv heads * G)
NT = D // 128                # 16 contraction tiles
NCHUNK = 4                   # token chunks of 512
CH = 512

_CACHE = {}


def _build():
    nc = bacc.Bacc("TRN2", target_bir_lowering=False, debug=False,
                   num_devices=N_CORES)
    f32, bf16 = mybir.dt.float32, mybir.dt.bfloat16

    xT = nc.dram_tensor("xT", [D, N], bf16, kind="ExternalInput")
    wq = nc.dram_tensor("wq", [D, 512], bf16, kind="ExternalInput")
    wk = nc.dram_tensor("wk", [D, 128], bf16, kind="ExternalInput")
    wv = nc.dram_tensor("wv", [D, 128], bf16, kind="ExternalInput")
    wo = nc.dram_tensor("wo", [512, D], bf16, kind="ExternalInput")
    msk = nc.dram_tensor("msk", [128, 256], bf16, kind="ExternalInput")
    part = nc.dram_tensor("part", [N, D], bf16, kind="ExternalOutput")

    with tile.TileContext(nc) as tc:
        with (
            tc.tile_pool(name="const", bufs=1) as cpool,
            tc.tile_pool(name="proj", bufs=1) as ppool,
            tc.tile_pool(name="work", bufs=6) as wpool,
            tc.tile_pool(name="att", bufs=1) as apool,
            tc.tile_pool(name="stage", bufs=4) as spool,
            tc.tile_pool(name="ps_s", bufs=2, space="PSUM") as ps_s,
            tc.tile_pool(name="ps_av", bufs=1, space="PSUM") as ps_av,
            tc.tile_pool(name="ps_m", bufs=2, space="PSUM") as ps_m,
        ):
            # ---- load constants / inputs to SBUF -------------------------
            # Input DMA is HBM-bandwidth bound (~37us for all inputs), so
            # order by first use: K/V weights, first xT token-window (so the
            # K projection starts ~8us in), Wq, remaining xT windows, Wo.
            xt = cpool.tile([128, NT * N], bf16, tag="xt")
            xtv = xt[:].rearrange("p (t n) -> p t n", t=NT)
            xsv = xT.ap().rearrange("(t p) n -> p t n", p=128)
            nc.sync.dma_start(xtv[:, :, 0:CH], xsv[:, :, 0:CH])
            wk_t = cpool.tile([128, NT * 128], bf16, tag="wk")
            nc.sync.dma_start(
                wk_t[:].rearrange("p (t o) -> p t o", t=NT),
                wk.ap().rearrange("(t p) o -> p t o", p=128))
            wv_t = cpool.tile([128, NT * 128], bf16, tag="wv")
            nc.sync.dma_start(
                wv_t[:].rearrange("p (t o) -> p t o", t=NT),
                wv.ap().rearrange("(t p) o -> p t o", p=128))
            msk_t = cpool.tile([128, 256], bf16, tag="msk")
            nc.sync.dma_start(msk_t[:], msk.ap()[:])
            wq_t = cpool.tile([128, NT * 512], bf16, tag="wq")
            nc.sync.dma_start(
                wq_t[:].rearrange("p (t o) -> p t o", t=NT),
                wq.ap().rearrange("(t p) o -> p t o", p=128))
            for j in range(1, 4):
                nc.sync.dma_start(
                    xtv[:, :, j * CH:(j + 1) * CH], xsv[:, :, j * CH:(j + 1) * CH])
            wo_t = cpool.tile([128, 4 * D], bf16, tag="wo")
            nc.sync.dma_start(
                wo_t[:].rearrange("p (t o) -> p t o", t=4),
                wo.ap().rearrange("(t p) o -> p t o", p=128))
            ones64 = cpool.tile([1, 64], f32, tag="ones64")
            nc.vector.memset(ones64[:], 1.0)

            # ---- projections --------------------------------------------
            # Emission is grouped by token-window j so the in-order PE queue
            # never blocks on a not-yet-DMA'd xT window while ready work for
            # an earlier window sits behind it.
            # kT2 [128 (2 kv heads x 64), N]
            kt2 = ppool.tile([128, N], bf16, tag="kt2")
            # v3: vT DMA-transposed per m-tile: 0:64 = V_h0, 64 = ones,
            # 65:129 = V_h1, 129 = ones (the ones columns produce the softmax
            # denominators in row 64 of the attn@V PSUM output).
            v3 = apool.tile([128, 16 * 130], bf16, tag="v3")
            nc.vector.memset(v3[:], 1.0)
            qt2 = []
            for a in range(4):
                qa = ppool.tile([128, N], bf16, tag=f"qt2_{a}")
                qt2.append(qa)
            for j in range(N // CH):
                ps = ps_m.tile([128, CH], f32, tag="misc")
                for t in range(NT):
                    nc.tensor.matmul(
                        ps[:], wk_t[:, t * 128:(t + 1) * 128],
                        xt[:, t * N + j * CH: t * N + (j + 1) * CH],
                        start=(t == 0), stop=(t == NT - 1))
                nc.scalar.activation(kt2[:, j * CH:(j + 1) * CH], ps[:],
                                     mybir.ActivationFunctionType.Copy)
                ps = ps_m.tile([128, CH], f32, tag="misc")
                for t in range(NT):
                    nc.tensor.matmul(
                        ps[:], wv_t[:, t * 128:(t + 1) * 128],
                        xt[:, t * N + j * CH: t * N + (j + 1) * CH],
                        start=(t == 0), stop=(t == NT - 1))
                vt_s = spool.tile([128, CH], bf16, tag="vt")
                nc.scalar.activation(vt_s[:], ps[:],
                                     mybir.ActivationFunctionType.Copy)
                for s in range(4):       # 4 m-tiles of 128 in this chunk
                    mt = 4 * j + s
                    # full-tile DMA transpose (partition-offset inputs are
                    # broken in the xbar path), then split around the ones
                    # column with two free-dim-offset DVE copies.
                    vtr = spool.tile([128, 128], bf16, tag="vtr")
                    nc.sync.dma_start_transpose(
                        vtr[:], vt_s[:, s * 128:(s + 1) * 128])
                    nc.vector.tensor_copy(v3[:, mt * 130: mt * 130 + 64],
                                          vtr[:, 0:64])
                    nc.vector.tensor_copy(v3[:, mt * 130 + 65: mt * 130 + 129],
                                          vtr[:, 64:128])
                # qT2 chunks a=0..3: [128 (head a | head a+4), N]
                for a in range(4):
                    ps = ps_m.tile([128, CH], f32, tag="misc")
                    for t in range(NT):
                        nc.tensor.matmul(
                            ps[:], wq_t[:, t * 512 + a * 128: t * 512 + (a + 1) * 128],
                            xt[:, t * N + j * CH: t * N + (j + 1) * CH],
                            start=(t == 0), stop=(t == NT - 1))
                    nc.scalar.activation(qt2[a][:, j * CH:(j + 1) * CH], ps[:],
                                         mybir.ActivationFunctionType.Copy)

            # ---- attention + o_proj per token chunk ---------------------
            for ci in range(NCHUNK):
                n0 = ci * CH
                mt_hi = 4 * ci + 4          # m-tiles 0..mt_hi-1
                aos = []                     # normalized attn outs per pair
                for a in range(4):
                    # pa: [0:64, 0:512]=head a+4 out, [0:64,512:1024]=head a
                    # out, row 64 of each half = softmax denominators.
                    # (h64 half goes in bank 0: walrus rejects tile_position
                    # row 64 combined with a non-zero PSUM output offset.)
                    pa = ps_av.tile([128, 2 * CH], f32, tag="av")
                    for mt in range(mt_hi):
                        diag = mt - 4 * ci
                        # columns left of the diagonal 128-block are fully
                        # masked for every m -> skip them in the score
                        # matmuls, the exp, and the attn@V streams.
                        lo = 128 * diag if diag > 0 else 0
                        ss = ps_s.tile([128, 2 * CH], f32, tag="s")
                        # two 64-contraction matmuls on distinct PE row
                        # groups (h64 / h0), issued adjacently -> concurrent
                        nc.tensor.matmul(
                            ss[:, lo:CH], kt2[64:128, mt * 128:(mt + 1) * 128],
                            qt2[a][64:128, n0 + lo:n0 + CH],
                            start=True, stop=True)
                        nc.tensor.matmul(
                            ss[:, CH + lo:2 * CH],
                            kt2[0:64, mt * 128:(mt + 1) * 128],
                            qt2[a][0:64, n0 + lo:n0 + CH],
                            start=True, stop=True)
                        pt = wpool.tile([128, 2 * CH], bf16, tag="pt")
                        if lo:
                            ssv = ss[:].rearrange("p (h n) -> p h n", h=2)
                            ptv = pt[:].rearrange("p (h n) -> p h n", h=2)
                            nc.scalar.activation(
                                ptv[:, :, lo:], ssv[:, :, lo:],
                                mybir.ActivationFunctionType.Exp, scale=0.125)
                        else:
                            nc.scalar.activation(
                                pt[:], ss[:],
                                mybir.ActivationFunctionType.Exp, scale=0.125)
                        if diag >= 0:
                            # only the 128-wide block containing the diagonal
                            # needs the triangular mask
                            ptv = pt[:].rearrange("p (h n) -> p h n", h=2)
                            mkv = msk_t[:].rearrange("p (h n) -> p h n", h=2)
                            nc.vector.tensor_mul(
                                ptv[:, :, lo:lo + 128], ptv[:, :, lo:lo + 128],
                                mkv)
                        nc.tensor.matmul(
                            pa[0:65, lo:CH],
                            v3[:, mt * 130 + 65: mt * 130 + 130],
                            pt[:, lo:CH],
                            start=(mt == 0), stop=(mt == mt_hi - 1),
                            skip_group_check=True)
                        nc.tensor.matmul(
                            pa[0:65, CH + lo:2 * CH],
                            v3[:, mt * 130: mt * 130 + 65],
                            pt[:, CH + lo:2 * CH],
                            start=(mt == 0), stop=(mt == mt_hi - 1),
                            skip_group_check=True)
                    # normalization: copy attn-out + denominators to SBUF
                    # right away (frees the pa PSUM banks for the next
                    # pair), then broadcast -> approx reciprocal -> scale.
                    sm = spool.tile([1, 2 * CH], f32, tag="sm")
                    nc.vector.tensor_copy(sm[:], pa[64:65, :])
                    rb = ps_m.tile([128, CH], f32, tag="misc")
                    nc.tensor.matmul(rb[0:64, :], ones64[0:1, :],
                                     sm[0:1, CH:2 * CH],
                                     start=True, stop=True, tile_position=(0, 0))
                    nc.tensor.matmul(rb[64:128, :], ones64[0:1, :],
                                     sm[0:1, 0:CH],
                                     start=True, stop=True, tile_position=(0, 64))
                    rbr = spool.tile([128, CH], f32, tag="rbr")
                    nc.vector.reciprocal_approx_fast(rbr[:], rb[:])
                    an = apool.tile([128, CH], bf16, tag=f"aos_{a}")
                    nc.vector.tensor_mul(an[0:64, :], pa[0:64, CH:2 * CH],
                                         rbr[0:64, :])
                    nc.vector.tensor_mul(an[64:128, :], pa[0:64, 0:CH],
                                         rbr[64:128, :])
                    aos.append(an)
                # o_proj: out[n, :] += sum_c attn_outT_s[c, n] * Wo[c, :]
                for nt in range(4):
                    for dc in range(4):
                        po = ps_m.tile([128, CH], f32, tag="misc")
                        for a in range(4):
                            nc.tensor.matmul(
                                po[:], aos[a][:, nt * 128:(nt + 1) * 128],
                                wo_t[:, a * D + dc * CH: a * D + (dc + 1) * CH],
                                start=(a == 0), stop=(a == 3))
                        st = spool.tile([128, CH], bf16, tag="ost")
                        nc.vector.tensor_copy(st[:], po[:])
                        nc.sync.dma_start(
                            part.ap()[n0 + nt * 128: n0 + (nt + 1) * 128,
                                      dc * CH:(dc + 1) * CH],
                            st[:])
    nc.compile()
    return nc


def _prep_in_maps(x, Wq, Wk, Wv, Wo):
    import jax.numpy as jnp

    def to_bf16(a):
        return np.asarray(jnp.asarray(np.asarray(a), dtype=jnp.bfloat16))

    # 128x128 triangular causal mask, duplicated side by side so one DVE
    # multiply covers both head-halves of the diagonal block of a pt tile.
    i = np.arange(128)[:, None]
    j = np.arange(128)[None, :]
    tri = (i <= j).astype(np.float32)
    msk = np.concatenate([tri, tri], axis=1)

    in_maps = []
    for c in range(N_CORES):
        b, g = c // 4, c % 4
        qh = [8 * g + a for a in range(8)]      # global q heads for this core
        # Wq columns reordered into pair chunks [head a | head a+4]
        wq_cols = []
        for a in range(4):
            wq_cols.append(np.arange(qh[a] * HD, (qh[a] + 1) * HD))
            wq_cols.append(np.arange(qh[a + 4] * HD, (qh[a + 4] + 1) * HD))
        wq_r = np.asarray(Wq)[:, np.concatenate(wq_cols)]
        wo_rows = wq_cols  # same ordering for Wo rows
        wo_r = np.asarray(Wo)[np.concatenate(wo_rows), :]
        wk_s = np.asarray(Wk)[:, 2 * g * HD: (2 * g + 2) * HD]
        wv_s = np.asarray(Wv)[:, 2 * g * HD: (2 * g + 2) * HD]
        in_maps.append({
            "xT": to_bf16(np.asarray(x)[b].T),
            "wq": to_bf16(wq_r),
            "wk": to_bf16(wk_s),
            "wv": to_bf16(wv_s),
            "wo": to_bf16(wo_r),
            "msk": to_bf16(msk),
        })
    return in_maps


def kernel(x, Wq, Wk, Wv, Wo, trace=False):
    if "nc" not in _CACHE:
        _CACHE["nc"] = _build()
    nc = _CACHE["nc"]
    in_maps = _prep_in_maps(x, Wq, Wk, Wv, Wo)
    res = bass_utils.run_bass_kernel_spmd(
        nc, in_maps, core_ids=list(range(N_CORES)), trace=trace)
    _CACHE["last_result"] = res
    out = np.zeros((B, N, D), np.float32)
    for c in range(N_CORES):
        out[c // 4] += np.asarray(res.results[c]["part"], dtype=np.float32)
    return out


# revision 38
# speedup vs baseline: 1.8226x; 1.0220x over previous
"""GroupedQueryAttention forward on 8 Trainium2 NeuronCores (Bass/Tile).

Sharding (per spec hint): data-parallel over batch (B=2) x tensor-parallel
over KV-head groups (4 groups of 2 KV heads + their 8 query heads each).
Core c -> (batch b = c // 4, group g = c % 4).

Each core computes, for its batch element and its 8 query heads:
  qT/kT projections in transposed layout (lhsT = W, rhs = xT), V natural via
  on-chip PE transpose of vT; causal softmax without max-subtraction (scores
  are ~N(0,1) after the 1/sqrt(hd) scale, exp cannot overflow); the softmax
  denominator is produced by the same matmul as attn@V via a ones-column
  appended to V.
  o_proj is row-parallel: each core emits a full [N, D] fp32 partial, and the
  host sums the 4 partials per batch element (the "all-reduce" of the o_proj).

Perf structure vs the v1 kernel:
  - the two 64-contraction score matmuls of a head pair write one 2-bank
    PSUM tile [128,1024] and are issued adjacently so the PE row-group
    tiling (h0 rows 0-63 / h64 rows 64-127) runs them concurrently;
  - exp runs once per (pair, mt) over [128,1024] (halves ACT instruction
    overhead), mask is one [128,1024] DVE multiply;
  - softmax normalization uses reciprocal_approx_fast and is fused into
    the PSUM->SBUF attn-out copy;
  - xT is DMA'd in 4 column windows so projections start immediately.

All device compute is bf16 with fp32 PSUM accumulation. The host pre-casts
and pre-transposes x (xT) and pre-slices/reorders the weight shards so the
device performs no layout work on the inputs.
"""

import numpy as np

import concourse.bass as bass  # noqa: F401  (import keeps engine registry warm)
import concourse.mybir as mybir
import concourse.tile as tile
from concourse import bacc, bass_utils

# Problem shape (hardcoded per contract).
B, N, D = 2, 2048, 2048
NUM_HEADS = 32
NUM_KV_HEADS = 8
HD = 64                      # head dim
G = NUM_HEADS // NUM_KV_HEADS  # 4 query heads per kv head
N_CORES = 8
LQ = 8                       # local query heads per core (2 kv heads * G)
NT = D // 128                # 16 contraction tiles
NCHUNK = 4                   # token chunks of 512
CH = 512

_CACHE = {}


def _build():
    nc = bacc.Bacc("TRN2", target_bir_lowering=False, debug=False,
                   num_devices=N_CORES)
    f32, bf16 = mybir.dt.float32, mybir.dt.bfloat16

    xT = nc.dram_tensor("xT", [D, N], bf16, kind="ExternalInput")
    wq = nc.dram_tensor("wq", [D, 512], bf16, kind="ExternalInput")
    wk = nc.dram_tensor("wk", [D, 128], bf16, kind="ExternalInput")
    wv = nc.dram_tensor("wv", [D, 128], bf16, kind="ExternalInput")
    wo = nc.dram_tensor("wo", [512, D], bf16, kind="ExternalInput")
    msk = nc.dram_tensor("msk", [128, 256], bf16, kind="ExternalInput")
    part = nc.dram_tensor("part", [N, D], bf16, kind="ExternalOutput")

    with tile.TileContext(nc) as tc:
        with (
            tc.tile_pool(name="const", bufs=1) as cpool,
            tc.tile_pool(name="proj", bufs=1) as ppool,
            tc.tile_pool(name="work", bufs=6) as wpool,
            tc.tile_pool(name="att", bufs=1) as apool,
            tc.tile_pool(name="attn_o", bufs=2) as aopool,
            tc.tile_pool(name="stage", bufs=4) as spool,
            tc.tile_pool(name="ps_s", bufs=2, space="PSUM") as ps_s,
            tc.tile_pool(name="ps_av", bufs=1, space="PSUM") as ps_av,
            tc.tile_pool(name="ps_m", bufs=2, space="PSUM") as ps_m,
        ):
            # ---- load constants / inputs to SBUF -------------------------
            # Input DMA is HBM-bandwidth bound (~37us for all inputs), so
            # order by first use: K/V weights, first xT token-window (so the
            # K projection starts ~8us in), Wq, remaining xT windows, Wo.
            xt = cpool.tile([128, NT * N], bf16, tag="xt")
            xtv = xt[:].rearrange("p (t n) -> p t n", t=NT)
            xsv = xT.ap().rearrange("(t p) n -> p t n", p=128)
            nc.sync.dma_start(xtv[:, :, 0:CH], xsv[:, :, 0:CH])
            wk_t = cpool.tile([128, NT * 128], bf16, tag="wk")
            nc.sync.dma_start(
                wk_t[:].rearrange("p (t o) -> p t o", t=NT),
                wk.ap().rearrange("(t p) o -> p t o", p=128))
            wv_t = cpool.tile([128, NT * 128], bf16, tag="wv")
            nc.sync.dma_start(
                wv_t[:].rearrange("p (t o) -> p t o", t=NT),
                wv.ap().rearrange("(t p) o -> p t o", p=128))
            msk_t = cpool.tile([128, 256], bf16, tag="msk")
            nc.sync.dma_start(msk_t[:], msk.ap()[:])
            wq_t = cpool.tile([128, NT * 512], bf16, tag="wq")
            nc.sync.dma_start(
                wq_t[:].rearrange("p (t o) -> p t o", t=NT),
                wq.ap().rearrange("(t p) o -> p t o", p=128))
            for j in range(1, 4):
                nc.sync.dma_start(
                    xtv[:, :, j * CH:(j + 1) * CH], xsv[:, :, j * CH:(j + 1) * CH])
            wo_t = cpool.tile([128, 4 * D], bf16, tag="wo")
            nc.sync.dma_start(
                wo_t[:].rearrange("p (t o) -> p t o", t=4),
                wo.ap().rearrange("(t p) o -> p t o", p=128))
            ones64 = cpool.tile([1, 64], f32, tag="ones64")
            nc.vector.memset(ones64[:], 1.0)

            # ---- projections --------------------------------------------
            # Emission is grouped by token-window j so the in-order PE queue
            # never blocks on a not-yet-DMA'd xT window while ready work for
            # an earlier window sits behind it.
            # kT2 [128 (2 kv heads x 64), N]
            kt2 = ppool.tile([128, N], bf16, tag="kt2")
            # v3: vT DMA-transposed per m-tile: 0:64 = V_h0, 64 = ones,
            # 65:129 = V_h1, 129 = ones (the ones columns produce the softmax
            # denominators in row 64 of the attn@V PSUM output).
            v3 = apool.tile([128, 16 * 130], bf16, tag="v3")
            nc.vector.memset(v3[:], 1.0)
            qt2 = []
            for a in range(4):
                qa = ppool.tile([128, N], bf16, tag=f"qt2_{a}")
                qt2.append(qa)
            for j in range(N // CH):
                ps = ps_m.tile([128, CH], f32, tag="misc")
                for t in range(NT):
                    nc.tensor.matmul(
                        ps[:], wk_t[:, t * 128:(t + 1) * 128],
                        xt[:, t * N + j * CH: t * N + (j + 1) * CH],
                        start=(t == 0), stop=(t == NT - 1))
                nc.scalar.activation(kt2[:, j * CH:(j + 1) * CH], ps[:],
                                     mybir.ActivationFunctionType.Copy)
                ps = ps_m.tile([128, CH], f32, tag="misc")
                for t in range(NT):
                    nc.tensor.matmul(
                        ps[:], wv_t[:, t * 128:(t + 1) * 128],
                        xt[:, t * N + j * CH: t * N + (j + 1) * CH],
                        start=(t == 0), stop=(t == NT - 1))
                vt_s = spool.tile([128, CH], bf16, tag="vt")
                nc.scalar.activation(vt_s[:], ps[:],
                                     mybir.ActivationFunctionType.Copy)
                for s in range(4):       # 4 m-tiles of 128 in this chunk
                    mt = 4 * j + s
                    # full-tile DMA transpose (partition-offset inputs are
                    # broken in the xbar path), then split around the ones
                    # column with two free-dim-offset DVE copies.
                    vtr = spool.tile([128, 128], bf16, tag="vtr")
                    nc.sync.dma_start_transpose(
                        vtr[:], vt_s[:, s * 128:(s + 1) * 128])
                    nc.vector.tensor_copy(v3[:, mt * 130: mt * 130 + 64],
                                          vtr[:, 0:64])
                    nc.vector.tensor_copy(v3[:, mt * 130 + 65: mt * 130 + 129],
                                          vtr[:, 64:128])
                # qT2 chunks a=0..3: [128 (head a | head a+4), N]
                for a in range(4):
                    ps = ps_m.tile([128, CH], f32, tag="misc")
                    for t in range(NT):
                        nc.tensor.matmul(
                            ps[:], wq_t[:, t * 512 + a * 128: t * 512 + (a + 1) * 128],
                            xt[:, t * N + j * CH: t * N + (j + 1) * CH],
                            start=(t == 0), stop=(t == NT - 1))
                    nc.scalar.activation(qt2[a][:, j * CH:(j + 1) * CH], ps[:],
                                         mybir.ActivationFunctionType.Copy)

            # ---- attention + o_proj per token chunk ---------------------
            # o_proj for chunk ci is EMITTED interleaved into chunk ci+1's
            # pair boundaries: its matmuls are ready work that fills the PE
            # queue while the normalization chain of a pair drains (the
            # in-order queue would otherwise stall on it).
            def emit_oproj(aos_p, n0_p, nt):
                for dc in range(4):
                    po = ps_m.tile([128, CH], f32, tag="misc")
                    for a4 in range(4):
                        nc.tensor.matmul(
                            po[:], aos_p[a4][:, nt * 128:(nt + 1) * 128],
                            wo_t[:, a4 * D + dc * CH: a4 * D + (dc + 1) * CH],
                            start=(a4 == 0), stop=(a4 == 3))
                    st = spool.tile([128, CH], bf16, tag="ost")
                    nc.vector.tensor_copy(st[:], po[:])
                    nc.sync.dma_start(
                        part.ap()[n0_p + nt * 128: n0_p + (nt + 1) * 128,
                                  dc * CH:(dc + 1) * CH],
                        st[:])

            prev = None                      # (aos, n0) of previous chunk
            for ci in range(NCHUNK):
                n0 = ci * CH
                mt_hi = 4 * ci + 4          # m-tiles 0..mt_hi-1
                aos = []                     # normalized attn outs per pair
                for a in range(4):
                    # pa: [0:64, 0:512]=head a+4 out, [0:64,512:1024]=head a
                    # out, row 64 of each half = softmax denominators.
                    # (h64 half goes in bank 0: walrus rejects tile_position
                    # row 64 combined with a non-zero PSUM output offset.)
                    pa = ps_av.tile([128, 2 * CH], f32, tag="av")
                    for mt in range(mt_hi):
                        diag = mt - 4 * ci
                        # columns left of the diagonal 128-block are fully
                        # masked for every m -> skip them in the score
                        # matmuls, the exp, and the attn@V streams.
                        lo = 128 * diag if diag > 0 else 0
                        ss = ps_s.tile([128, 2 * CH], f32, tag="s")
                        # two 64-contraction matmuls on distinct PE row
                        # groups (h64 / h0), issued adjacently -> concurrent
                        nc.tensor.matmul(
                            ss[:, lo:CH], kt2[64:128, mt * 128:(mt + 1) * 128],
                            qt2[a][64:128, n0 + lo:n0 + CH],
                            start=True, stop=True)
                        nc.tensor.matmul(
                            ss[:, CH + lo:2 * CH],
                            kt2[0:64, mt * 128:(mt + 1) * 128],
                            qt2[a][0:64, n0 + lo:n0 + CH],
                            start=True, stop=True)
                        pt = wpool.tile([128, 2 * CH], bf16, tag="pt")
                        if lo:
                            ssv = ss[:].rearrange("p (h n) -> p h n", h=2)
                            ptv = pt[:].rearrange("p (h n) -> p h n", h=2)
                            nc.scalar.activation(
                                ptv[:, :, lo:], ssv[:, :, lo:],
                                mybir.ActivationFunctionType.Exp, scale=0.125)
                        else:
                            nc.scalar.activation(
                                pt[:], ss[:],
                                mybir.ActivationFunctionType.Exp, scale=0.125)
                        if diag >= 0:
                            # only the 128-wide block containing the diagonal
                            # needs the triangular mask
                            ptv = pt[:].rearrange("p (h n) -> p h n", h=2)
                            mkv = msk_t[:].rearrange("p (h n) -> p h n", h=2)
                            nc.vector.tensor_mul(
                                ptv[:, :, lo:lo + 128], ptv[:, :, lo:lo + 128],
                                mkv)
                        nc.tensor.matmul(
                            pa[0:65, lo:CH],
                            v3[:, mt * 130 + 65: mt * 130 + 130],
                            pt[:, lo:CH],
                            start=(mt == 0), stop=(mt == mt_hi - 1),
                            skip_group_check=True)
                        nc.tensor.matmul(
                            pa[0:65, CH + lo:2 * CH],
                            v3[:, mt * 130: mt * 130 + 65],
                            pt[:, CH + lo:2 * CH],
                            start=(mt == 0), stop=(mt == mt_hi - 1),
                            skip_group_check=True)
                    # normalization: copy attn-out + denominators to SBUF
                    # right away (frees the pa PSUM banks for the next
                    # pair), then broadcast -> approx reciprocal -> scale.
                    sm = spool.tile([1, 2 * CH], f32, tag="sm")
                    nc.vector.tensor_copy(sm[:], pa[64:65, :])
                    rb = ps_m.tile([128, CH], f32, tag="misc")
                    nc.tensor.matmul(rb[0:64, :], ones64[0:1, :],
                                     sm[0:1, CH:2 * CH],
                                     start=True, stop=True, tile_position=(0, 0))
                    nc.tensor.matmul(rb[64:128, :], ones64[0:1, :],
                                     sm[0:1, 0:CH],
                                     start=True, stop=True, tile_position=(0, 64))
                    rbr = spool.tile([128, CH], f32, tag="rbr")
                    nc.vector.reciprocal_approx_fast(rbr[:], rb[:])
                    an = aopool.tile([128, CH], bf16, tag=f"aos_{a}")
                    nc.vector.tensor_mul(an[0:64, :], pa[0:64, CH:2 * CH],
                                         rbr[0:64, :])
                    nc.vector.tensor_mul(an[64:128, :], pa[0:64, 0:CH],
                                         rbr[64:128, :])
                    aos.append(an)
                    if prev is not None:
                        emit_oproj(prev[0], prev[1], a)
                prev = (aos, n0)
            # last chunk's o_proj has no following pairs to hide behind
            for nt in range(4):
                emit_oproj(prev[0], prev[1], nt)
    nc.compile()
    return nc


def _prep_in_maps(x, Wq, Wk, Wv, Wo):
    import jax.numpy as jnp

    def to_bf16(a):
        return np.asarray(jnp.asarray(np.asarray(a), dtype=jnp.bfloat16))

    # 128x128 triangular causal mask, duplicated side by side so one DVE
    # multiply covers both head-halves of the diagonal block of a pt tile.
    i = np.arange(128)[:, None]
    j = np.arange(128)[None, :]
    tri = (i <= j).astype(np.float32)
    msk = np.concatenate([tri, tri], axis=1)

    in_maps = []
    for c in range(N_CORES):
        b, g = c // 4, c % 4
        qh = [8 * g + a for a in range(8)]      # global q heads for this core
        # Wq columns reordered into pair chunks [head a | head a+4]
        wq_cols = []
        for a in range(4):
            wq_cols.append(np.arange(qh[a] * HD, (qh[a] + 1) * HD))
            wq_cols.append(np.arange(qh[a + 4] * HD, (qh[a + 4] + 1) * HD))
        wq_r = np.asarray(Wq)[:, np.concatenate(wq_cols)]
        wo_rows = wq_cols  # same ordering for Wo rows
        wo_r = np.asarray(Wo)[np.concatenate(wo_rows), :]
        wk_s = np.asarray(Wk)[:, 2 * g * HD: (2 * g + 2) * HD]
        wv_s = np.asarray(Wv)[:, 2 * g * HD: (2 * g + 2) * HD]
        in_maps.append({
            "xT": to_bf16(np.asarray(x)[b].T),
            "wq": to_bf16(wq_r),
            "wk": to_bf16(wk_s),
            "wv": to_bf16(wv_s),
            "wo": to_bf16(wo_r),
            "msk": to_bf16(msk),
        })
    return in_maps


def kernel(x, Wq, Wk, Wv, Wo, trace=False):
    if "nc" not in _CACHE:
        _CACHE["nc"] = _build()
    nc = _CACHE["nc"]
    in_maps = _prep_in_maps(x, Wq, Wk, Wv, Wo)
    res = bass_utils.run_bass_kernel_spmd(
        nc, in_maps, core_ids=list(range(N_CORES)), trace=trace)
    _CACHE["last_result"] = res
    out = np.zeros((B, N, D), np.float32)
    for c in range(N_CORES):
        out[c // 4] += np.asarray(res.results[c]["part"], dtype=np.float32)
    return out
